# revision 25
# baseline (speedup 1.0000x reference)
"""Trainium2 Bass kernel for nn_GAT_edge: dilated ResNet-101 + 3-layer edge-GAT.

Parallelization: 2 images x 4-way spatial split over H (8 NeuronCores).
Halo exchange per bottleneck block via AllGather + indirect-DMA gathers.
GAT: edges partitioned by destination-node quarter; per-layer node-feature
AllGather. Final outputs: per-core row slices, reassembled on host.

kernel(**inputs) -> tuple of 5 np.ndarrays matching reference.py.
"""
import numpy as np
import ml_dtypes
from contextlib import ExitStack

import concourse.bass as bass
import concourse.tile as tile
from concourse import bacc, mybir
from concourse.bass_utils import run_bass_kernel_spmd

FP32 = mybir.dt.float32
BF16 = mybir.dt.bfloat16
INT32 = mybir.dt.int32
AF = mybir.ActivationFunctionType
ALU = mybir.AluOpType

CONV_DT = FP32          # BF16 | FP32 | mybir.dt.float32r
EPS = 1e-5
N_CORES = 8
RG = [[0, 1, 2, 3], [4, 5, 6, 7]]
N_NODES = 1280
STAGES = [(64, 3, 1, 1), (128, 4, 2, 1), (256, 23, 1, 2), (512, 3, 1, 4)]
GAT_CFG = [(128, 3, 64, 1, 3), (192, 3, 32, 1, 3), (96, 3, 32, 1, 1)]
TBLW = 204
TBL_GM = 200
SENT = 1 << 20          # OOB sentinel for halo gathers at image edges

DEBUG = set()           # e.g. {"stem", "s1", "s2", "s3", "s4", "nf0"}
UPTO = "all"            # stem|s1|s2|s3|s4|red|gat|all


def np_dt(dt):
    return ml_dtypes.bfloat16 if dt == BF16 else np.float32


def _np(x):
    return np.asarray(x)


# ================================================================ host packing

class Plane:
    """[128, LEN] plane; blocks at (cols off:off+M, partitions 0:K)."""

    def __init__(self, dt):
        self.cols, self.len, self.index, self.dt = [], 0, {}, dt
        self.gindex = {}

    def add(self, name, blkKM):
        Kd, M = blkKM.shape
        assert Kd <= 128, (name, blkKM.shape)
        blk = np.zeros((128, M), dtype=np_dt(self.dt))
        blk[:Kd] = blkKM.astype(np_dt(self.dt))
        off = self.len
        self.cols.append(blk)
        self.len += M
        self.index[name] = (off, Kd, M)
        return off

    def plane(self):
        if not self.cols:
            return np.zeros((128, 1), dtype=np_dt(self.dt))
        return np.concatenate(self.cols, axis=1)


def tiles_of(C):
    return [min(128, C - c) for c in range(0, C, 128)]


def fold_bn(w, bn):
    g, b, m, v = (_np(bn[x]).astype(np.float64) for x in ('g', 'b', 'm', 'v'))
    s = g / np.sqrt(v + EPS)
    return (_np(w).astype(np.float64) * s[:, None, None, None]).astype(np.float32), \
        (b - m * s).astype(np.float32)


def pack_1x1(wp, bp, name, w, bias):
    O, I = w.shape[:2]
    w = w.reshape(O, I)
    kts, mts = tiles_of(I), tiles_of(O)
    for mi in range(len(mts)):
        g0 = wp.len
        for ki in range(len(kts)):
            wp.add(f"{name}.{ki}.{mi}",
                   w[mi * 128:mi * 128 + mts[mi], ki * 128:ki * 128 + kts[ki]].T)
        wp.gindex[(name, mi)] = (g0, wp.len - g0)
    if bias is not None:
        bp_add(bp, name, bias, mts)
    return dict(kts=kts, mts=mts)


def pack_3x3(wp, bp, name, w, bias):
    O, I = w.shape[:2]
    kts, mts = tiles_of(I), tiles_of(O)
    for mi in range(len(mts)):
        g0 = wp.len
        for dy in range(3):
            for dx in range(3):
                for ki in range(len(kts)):
                    wp.add(f"{name}.{dy}{dx}.{ki}.{mi}",
                           w[mi * 128:mi * 128 + mts[mi],
                             ki * 128:ki * 128 + kts[ki], dy, dx].T)
        wp.gindex[(name, mi)] = (g0, wp.len - g0)
    bp_add(bp, name, bias, mts)
    return dict(kts=kts, mts=mts)


def bp_add(bp, name, bias, mts):
    cols = []
    for mi, ms in enumerate(mts):
        col = np.zeros((128, 1), np.float32)
        col[:ms, 0] = bias[mi * 128:mi * 128 + ms]
        cols.append(col)
    blk = np.concatenate(cols, axis=1)
    off = bp.len
    bp.cols.append(blk)
    bp.len += blk.shape[1]
    bp.index[name] = (off, 128, blk.shape[1])


# ================================================================ device utils

def v3(t, P, base, R, Wp):
    """view [P, R, Wp] of flat tile t at col offset base"""
    return t[:P, base:base + R * Wp].rearrange("p (r w) -> p r w", w=Wp)


class K:
    def __init__(self, nc, tc, ctx, H, io):
        self.nc, self.tc, self.ctx, self.H, self.io = nc, tc, ctx, H, io
        self.pools = {}
        self.gblk = 0

    def pool(self, name, bufs=1, space="SBUF"):
        if name not in self.pools:
            self.pools[name] = self.ctx.enter_context(
                self.tc.tile_pool(name=name, bufs=bufs, space=space))
        return self.pools[name]

    def psum(self, n, tag="ps"):
        return self.pool("psum", bufs=8, space="PSUM").tile(
            [128, n], FP32, tag="ps", name="pst")


def load_plane(k, pool_name, io_name, dt, tag=None, bufs=1):
    t = k.pool(pool_name, bufs=bufs).tile(list(k.io[io_name].shape), dt,
                                          tag=tag or io_name)
    k.nc.sync.dma_start(t[:], k.io[io_name][:])
    return t


def dbg_out(k, name, src_ap, shape):
    """declare debug output and write src (cast to fp32)"""
    nc = k.nc
    o = nc.dram_tensor(f"dbg_{name}", list(shape), FP32, kind="ExternalOutput")
    tmp = k.pool("dbgp", bufs=2).tile(list(shape), FP32, tag="dbg")
    nc.vector.tensor_copy(tmp[:src_ap.shape[0]], src_ap)
    nc.sync.dma_start(o[:src_ap.shape[0]], tmp[:src_ap.shape[0]])


# ================================================================ stem

def emit_stem(k):
    nc, H, io = k.nc, k.H, k.io
    wp, bp = H['wp_stem'], H['bp_stem']
    mk = load_plane(k, "consts", "masks", FP32)
    cd = carry_dram(k, "carry_stem", 16 * 80)

    with ExitStack() as sctx:
        act = sctx.enter_context(k.tc.tile_pool(name="stem_act", bufs=1))
        wpool = sctx.enter_context(k.tc.tile_pool(name="stem_w", bufs=1))
        xpool = sctx.enter_context(k.tc.tile_pool(name="stem_x", bufs=3))
        wt = wpool.tile([128, wp.len], CONV_DT, tag="w_stem", name="w_stem")
        nc.sync.dma_start(wt[:], io['w_stem'][:])
        bt = wpool.tile([128, bp.len], FP32, tag="b_stem", name="b_stem")
        nc.sync.dma_start(bt[:], io['b_stem'][:])

        R1, W1, Wp1 = 71, 320, 322
        c1 = act.tile([128, R1 * Wp1], CONV_DT, tag="c1buf", name="c1buf")
        nc.gpsimd.memset(c1[:64], 0.0)
        woff = wp.index['c1'][0]
        boff = bp.index['c1'][0]
        c1v = v3(c1, 64, 0, R1, Wp1)
        CH = 4
        for r0 in range(0, R1, CH):
            nr = min(CH, R1 - r0)
            xt = xpool.tile([128, CH * W1], CONV_DT, tag="x27c", name="x27c")
            nc.sync.dma_start(xt[:27, :nr * W1],
                              io['x27'][:, r0 * W1:(r0 + nr) * W1])
            for r in range(nr):
                ps = k.psum(W1)
                nc.tensor.matmul(ps[:64], wt[:27, woff:woff + 64],
                                 xt[:27, r * W1:(r + 1) * W1], start=True, stop=True)
                nc.scalar.activation(c1v[:, r0 + r, 1:321], ps[:64], AF.Relu,
                                     bias=bt[:64, boff:boff + 1])
        nc.vector.tensor_scalar_mul(c1v[:, 2, :], c1v[:, 2, :], mk[:64, 0:1])

        # conv2 64->64 k3 s2: out slot i reads c1 slots 2i+dy, col 2c+dx
        R2, W2, Wp2 = 35, 160, 162
        c2 = act.tile([128, R2 * Wp2], CONV_DT, tag="c2buf", name="c2buf")
        nc.gpsimd.memset(c2[:64], 0.0)
        boff2 = bp.index['c2'][0]
        c2v = v3(c2, 64, 0, R2, Wp2)
        for r0 in range(0, R2, 3):
            nr = min(3, R2 - r0)
            ps = k.psum(nr * W2)
            acc = 0
            for dy in range(3):
                for dx in range(3):
                    wo = wp.index[f'c2.{dy}{dx}'][0]
                    rhs = c1v[:, 2 * r0 + dy: 2 * r0 + dy + 2 * nr - 1:2,
                              dx:dx + 2 * W2 - 1:2]
                    nc.tensor.matmul(ps[:64], wt[:64, wo:wo + 64], rhs,
                                     start=(acc == 0), stop=(acc == 8))
                    acc += 1
            nc.scalar.activation(c2v[:, r0:r0 + nr, 1:161],
                                 ps[:64].rearrange("p (r w) -> p r w", r=nr),
                                 AF.Relu, bias=bt[:64, boff2:boff2 + 1])
        nc.vector.tensor_scalar_mul(c2v[:, 0, :], c2v[:, 0, :], mk[:64, 1:2])
        nc.vector.tensor_scalar_mul(c2v[:, 33, :], c2v[:, 33, :], mk[:64, 2:3])
        nc.vector.tensor_scalar_mul(c2v[:, 34, :], c2v[:, 34, :], mk[:64, 3:4])

        # conv3 64->128 k3 p1: out slot i reads c2 slots i+dy, col c+dx
        R3, W3, Wp3 = 33, 160, 162
        c3 = act.tile([128, R3 * Wp3], CONV_DT, tag="c3buf", name="c3buf")
        nc.gpsimd.memset(c3[:], 0.0)
        boff3 = bp.index['c3'][0]
        c3v = v3(c3, 128, 0, R3, Wp3)
        for r0 in range(0, R3, 3):
            nr = min(3, R3 - r0)
            ps = k.psum(nr * W3)
            acc = 0
            for dy in range(3):
                for dx in range(3):
                    wo = wp.index[f'c3.{dy}{dx}'][0]
                    rhs = c2v[:, r0 + dy: r0 + dy + nr, dx:dx + W3]
                    nc.tensor.matmul(ps[:, :], wt[:64, wo:wo + 128], rhs,
                                     start=(acc == 0), stop=(acc == 8))
                    acc += 1
            nc.scalar.activation(c3v[:, r0:r0 + nr, 1:161],
                                 ps.rearrange("p (r w) -> p r w", r=nr),
                                 AF.Relu, bias=bt[:, boff3:boff3 + 1])
        nc.vector.tensor_scalar_mul(c3v[:, 32, :], c3v[:, 32, :], mk[:, 4:5])

        # feats0 = c3 rows [0:32)
        if CONV_DT == FP32:
            nc.sync.dma_start(io['feats0'].rearrange("p (r w) -> p r w", w=160),
                              c3v[:, 0:32, 1:161])
        else:
            f0 = act.tile([128, 32 * 160], FP32, tag="f0", name="f0")
            nc.vector.tensor_copy(f0.rearrange("p (r w) -> p r w", r=32),
                                  c3v[:, 0:32, 1:161])
            nc.sync.dma_start(io['feats0'][:], f0[:])

        # maxpool k3 s2 (ceil)
        pooled = act.tile([128, 16 * 80], CONV_DT, tag="pooled", name="pooled")
        pv = pooled.rearrange("p (r w) -> p r w", r=16)
        first = True
        for dy in range(3):
            for dx in range(3):
                src = c3v[:, dy:dy + 31:2, 1 + dx:1 + dx + 159:2]
                if first:
                    nc.vector.tensor_copy(pv, src)
                    first = False
                else:
                    nc.vector.tensor_tensor(out=pv, in0=pv, in1=src, op=ALU.max)
        nc.sync.dma_start(cd[:], pooled[:])
        if "stem" in DEBUG:
            dbg_out(k, "pooled", pooled[:], (128, 16 * 80))
    return "carry_stem"


def emit_block(k, sname, bi, geo, x, y, wpool, bt, cc_in, cc_out, hidx, resources):
    """One bottleneck block with streamed per-(conv, m-tile) weights."""
    nc, H = k.nc, k.H
    wp, bp = H[f'wp_{sname}'], H[f'bp_{sname}']
    wdram = k.io[f'w_{sname}']
    wbase = bi * H[f'wlen_{sname}']
    Cin, Cm, Cout = geo['Cin'], geo['Cm'], geo['Cout']
    d, W, rows = geo['d'], geo['W'], geo['rows']
    inW, inRows, stride = geo['inW'], geo['inRows'], geo['stride']
    first = geo['first']
    kti, ktm, kto = tiles_of(Cin), tiles_of(Cm), tiles_of(Cout)
    name = f"{sname}b{bi}"
    pb, c2o, resid = resources['pb'], resources['c2o'], resources['resid']

    PR, PW = inRows + 2 * d, inW + 2 * d
    npx_in = inRows * inW
    npx = rows * W

    def wload(cname, mi, tag):
        off, ln = wp.gindex[(f'{name}.{cname}', mi)]
        t = wpool.tile([128, ln], CONV_DT, tag=tag, name=f"w_{tag}")
        nc.sync.dma_start(t[:], wdram[:, wbase + off: wbase + off + ln])
        return t

    # --- conv1 (1x1 at input res) -> pb interior
    bo1 = bp.index[f'{name}.c1'][0]
    chunk = max(1, 512 // inW) * inW
    for mi, ms in enumerate(ktm):
        wt1 = wload('c1', mi, 'wc1')
        c0 = 0
        while c0 < npx_in:
            n = min(chunk, npx_in - c0)
            nr, r0 = n // inW, c0 // inW
            ps = k.psum(n)
            for ki, ks in enumerate(kti):
                nc.tensor.matmul(ps[:ms], wt1[:ks, ki * ms:(ki + 1) * ms],
                                 x[:ks, ki * npx_in + c0: ki * npx_in + c0 + n],
                                 start=(ki == 0), stop=(ki == len(kti) - 1))
            pbv = v3(pb, ms, mi * PR * PW, PR, PW)
            nc.scalar.activation(pbv[:, d + r0: d + r0 + nr, d:d + inW],
                                 ps[:ms].rearrange("p (r w) -> p r w", r=nr),
                                 AF.Relu, bias=bt[:ms, bo1 + mi:bo1 + mi + 1])
            c0 += n

    # --- halo exchange on conv1 output
    for mi, ms in enumerate(ktm):
        pbv = v3(pb, ms, mi * PR * PW, PR, PW)
        nc.sync.dma_start(cc_in[mi * 128:mi * 128 + ms, 0], pbv[:, d:2 * d, d:d + inW])
        nc.sync.dma_start(cc_in[mi * 128:mi * 128 + ms, 1],
                          pbv[:, inRows:inRows + d, d:d + inW])
    nc.gpsimd.collective_compute("AllGather", ALU.bypass, replica_groups=RG,
                                 ins=[cc_in[:]], outs=[cc_out[:]])
    ccf = cc_out.rearrange("g c s d w -> (g c s) (d w)")
    nunits = 4 * Cm * 2
    hsc = resources['hsc']
    hw = d * inW
    for mi, ms in enumerate(ktm):
        pbv = v3(pb, ms, mi * PR * PW, PR, PW)
        sc_t = hsc[:ms, (2 * mi) * hw:(2 * mi + 1) * hw]
        sc_b = hsc[:ms, (2 * mi + 1) * hw:(2 * mi + 2) * hw]
        nc.gpsimd.indirect_dma_start(
            sc_t, None, ccf[:],
            bass.IndirectOffsetOnAxis(ap=hidx[:ms, 2 * mi:2 * mi + 1], axis=0),
            bounds_check=nunits - 1, oob_is_err=False)
        nc.gpsimd.indirect_dma_start(
            sc_b, None, ccf[:],
            bass.IndirectOffsetOnAxis(ap=hidx[:ms, 2 * mi + 1:2 * mi + 2], axis=0),
            bounds_check=nunits - 1, oob_is_err=False)
        nc.vector.tensor_copy(pbv[:, 0:d, d:d + inW],
                              sc_t.rearrange("p (d w) -> p d w", d=d))
        nc.vector.tensor_copy(pbv[:, inRows + d:inRows + 2 * d, d:d + inW],
                              sc_b.rearrange("p (d w) -> p d w", d=d))

    # --- conv2 (3x3 dil d, stride)
    bo2 = bp.index[f'{name}.c2'][0]
    nkt = len(ktm)
    crows = max(1, 512 // W)
    for mi, ms in enumerate(ktm):
        wt2 = wload('c2', mi, 'wc2')
        for r0 in range(0, rows, crows):
            nr = min(crows, rows - r0)
            ps = k.psum(nr * W)
            acc, nacc = 0, 9 * nkt
            for dy in range(3):
                for dx in range(3):
                    rb, cb = dy * d + r0 * stride, dx * d
                    for ki, ks in enumerate(ktm):
                        lo = ((dy * 3 + dx) * nkt + ki) * ms
                        pbv = v3(pb, ks, ki * PR * PW, PR, PW)
                        rhs = pbv[:, rb:rb + (nr - 1) * stride + 1:stride,
                                  cb:cb + (W - 1) * stride + 1:stride]
                        nc.tensor.matmul(ps[:ms], wt2[:ks, lo:lo + ms], rhs,
                                         start=(acc == 0), stop=(acc == nacc - 1))
                        acc += 1
            nc.scalar.activation(c2o[:ms, mi * npx + r0 * W: mi * npx + (r0 + nr) * W],
                                 ps[:ms], AF.Relu, bias=bt[:ms, bo2 + mi:bo2 + mi + 1])

    # --- downsample residual (block 1)
    if first:
        bod = bp.index[f'{name}.ds'][0]
        crows = max(1, 512 // W)
        for mi, ms in enumerate(kto):
            wtd = wload('ds', mi, 'wcd')
            for r0 in range(0, rows, crows):
                nr = min(crows, rows - r0)
                ps = k.psum(nr * W)
                for ki, ks in enumerate(kti):
                    if stride == 1:
                        rhs = x[:ks, ki * npx_in + r0 * W: ki * npx_in + (r0 + nr) * W]
                    else:
                        xv = v3(x, ks, ki * npx_in, inRows, inW)
                        rhs = xv[:, r0 * stride:(r0 + nr - 1) * stride + 1:stride,
                                 0:inW - stride + 1:stride]
                    nc.tensor.matmul(ps[:ms], wtd[:ks, ki * ms:(ki + 1) * ms], rhs,
                                     start=(ki == 0), stop=(ki == len(kti) - 1))
                nc.scalar.activation(
                    resid[:ms, mi * npx + r0 * W: mi * npx + (r0 + nr) * W],
                    ps[:ms], AF.Identity, bias=bt[:ms, bod + mi:bod + mi + 1])
        res_t, res_F = resid, npx
    else:
        res_t, res_F = x, npx_in

    # --- conv3 (1x1) + residual + relu -> y
    bo3 = bp.index[f'{name}.c3'][0]
    for mi, ms in enumerate(kto):
        wt3 = wload('c3', mi, 'wc3')
        for c0 in range(0, npx, 512):
            n = min(512, npx - c0)
            ps = k.psum(n)
            for ki, ks in enumerate(ktm):
                nc.tensor.matmul(ps[:ms], wt3[:ks, ki * ms:(ki + 1) * ms],
                                 c2o[:ks, ki * npx + c0: ki * npx + c0 + n],
                                 start=(ki == 0), stop=(ki == len(ktm) - 1))
            ys = y[:ms, mi * npx + c0: mi * npx + c0 + n]
            nc.vector.tensor_tensor(out=ys, in0=ps[:ms],
                                    in1=res_t[:ms, mi * res_F + c0: mi * res_F + c0 + n],
                                    op=ALU.add)
            nc.scalar.activation(ys, ys, AF.Relu, bias=bt[:ms, bo3 + mi:bo3 + mi + 1])

    if k.H.get('tapblk') == (sname, bi):
        KTm, KTo = len(ktm), len(kto)
        o1 = nc.dram_tensor("dbg_ccin", [Cm * 2 * d * inW], FP32, kind="ExternalOutput")
        nc.sync.dma_start(o1[:], cc_in.rearrange("c s d w -> (c s d w)"))
        o2 = nc.dram_tensor("dbg_ccout", [4 * Cm * 2 * d * inW], FP32,
                            kind="ExternalOutput")
        nc.sync.dma_start(o2[:], cc_out.rearrange("g c s d w -> (g c s d w)"))
        dbg_out(k, "pb", pb[:], (128, KTm * PR * PW))
        dbg_out(k, "c2o", c2o[:], (128, KTm * npx))
        if first:
            dbg_out(k, "resid", resid[:], (128, KTo * npx))
        dbg_out(k, "yblk", y[:], (128, KTo * npx))


def carry_dram(k, name, n):
    if name not in k.io:
        k.io[name] = k.nc.dram_tensor(name, [128, n], CONV_DT)
    return k.io[name]


def emit_stage(k, sname, blocks_geo, carry_in, feats_name=None):
    """Run blocks of a stage in a scoped pool; returns output carry name."""
    nc, H, io = k.nc, k.H, k.io
    gN = blocks_geo[-1]
    Cm = blocks_geo[0]['Cm']
    ktm, kto = tiles_of(Cm), tiles_of(gN['Cout'])
    rows, W = gN['rows'], gN['W']
    npx = rows * W

    with ExitStack() as sctx:
        act = sctx.enter_context(k.tc.tile_pool(name=f"{sname}_act", bufs=1))
        wpool = sctx.enter_context(k.tc.tile_pool(name=f"{sname}_w", bufs=2))

        geoms = {}
        for geo in blocks_geo:
            key = (geo['inRows'], geo['inW'], geo['d'])
            if key not in geoms:
                gi = len(geoms)
                PR, PW = geo['inRows'] + 2 * geo['d'], geo['inW'] + 2 * geo['d']
                pb = act.tile([128, len(ktm) * PR * PW], CONV_DT,
                              tag=f"{sname}_pb{gi}", name=f"pb_{sname}{gi}")
                nc.gpsimd.memset(pb[:], 0.0)
                hscg = act.tile([128, 2 * len(ktm) * geo['d'] * geo['inW']], CONV_DT,
                                tag=f"{sname}_hsc{gi}", name=f"hsc_{sname}{gi}")
                nc.gpsimd.memset(hscg[:], 0.0)
                ccs = []
                for par in range(2):
                    ci = nc.dram_tensor(f"cc_{sname}_{gi}_{par}_in",
                                        [Cm, 2, geo['d'], geo['inW']], CONV_DT)
                    co = nc.dram_tensor(f"cc_{sname}_{gi}_{par}_out",
                                        [4, Cm, 2, geo['d'], geo['inW']], CONV_DT)
                    ccs.append((ci, co))
                geoms[key] = (pb, ccs, hscg)

        c2o = act.tile([128, len(ktm) * npx], CONV_DT, tag=f"{sname}_c2o",
                       name=f"c2o_{sname}")
        resid = act.tile([128, len(kto) * npx], CONV_DT, tag=f"{sname}_res",
                         name=f"res_{sname}")
        hidx = k.pool("consts").tile([128, 2 * len(ktm)], INT32,
                                     tag=f"hidx_{sname}", name=f"hx_{sname}")
        nc.sync.dma_start(hidx[:], io[f'hidx_{sname}'][:])
        bt = k.pool("consts").tile([128, H[f'bp_{sname}'].len], FP32,
                                   tag=f"b_{sname}", name=f"bt_{sname}")
        nc.sync.dma_start(bt[:], io[f'b_{sname}'][:])

        kin = tiles_of(blocks_geo[0]['Cin'])
        npx_in0 = blocks_geo[0]['inRows'] * blocks_geo[0]['inW']
        x = act.tile([128, len(kin) * npx_in0], CONV_DT, tag=f"{sname}_x0",
                     name=f"x0_{sname}")
        nc.sync.dma_start(x[:], k.io[carry_in][:])
        ya = act.tile([128, len(kto) * npx], CONV_DT, tag=f"{sname}_ya",
                      name=f"ya_{sname}")
        yb = act.tile([128, len(kto) * npx], CONV_DT, tag=f"{sname}_yb",
                      name=f"yb_{sname}")
        cd = carry_dram(k, f"carry_{sname}", len(kto) * npx)
        nlim = k.H.get(f'nblk_{sname}')
        if nlim:
            blocks_geo = blocks_geo[:nlim]
        for bi, geo in enumerate(blocks_geo):
            ytile = ya if bi % 2 == 0 else yb
            pb, ccs, hsc = geoms[(geo['inRows'], geo['inW'], geo['d'])]
            ci, co = ccs[bi % 2]
            res = dict(pb=pb, c2o=c2o, resid=resid, hsc=hsc)
            emit_block(k, sname, bi, geo, x, ytile, wpool, bt, ci, co, hidx, res)
            x = ytile

        if feats_name is not None:
            if CONV_DT == FP32:
                nc.sync.dma_start(io[feats_name][:], x[:])
            else:
                f = act.tile([128, len(kto) * npx], FP32, tag=f"{sname}_f",
                             name=f"f_{sname}")
                nc.vector.tensor_copy(f[:], x[:])
                nc.sync.dma_start(io[feats_name][:], f[:])
        nc.sync.dma_start(cd[:], x[:])
        if sname in DEBUG:
            dbg_out(k, sname, x[:], (128, len(kto) * npx))
    return f"carry_{sname}"


def emit_gat(k, nf0):
    """nf0: [128, 1280] fp32 channel-major full-image node features."""
    nc, H, io = k.nc, k.H, k.io
    NSLOT = H['NSLOT']
    NCH = 3 * NSLOT
    gact = k.pool("gat", bufs=1)
    wg = load_plane(k, "gat_w", "w_gat", FP32)
    eye = load_plane(k, "consts", "eye", FP32)
    St = load_plane(k, "gat_w", "S", FP32)
    gidx = load_plane(k, "consts", "gidx", INT32)

    table = nc.dram_tensor("gtable", [N_NODES, TBLW], FP32)
    nc.sync.dma_start(table[:, TBL_GM:TBL_GM + 3], io['gm'][:])

    ag_prev = None   # DRAM [4, 384, F_prev] from previous layer
    fout_prev = None
    for li, (inn, ine, on, oe, h) in enumerate(GAT_CFG):
        Mw = on * h
        Mb = Mw + 2 * h
        Fs = Mw + h
        Fg = TBLW  # full-row gathers (indirect coef = row width)
        ktn = tiles_of(inn)
        wfij = H['gat_wfij'][li]      # [h, 3]
        attn = H['gat_attn'][li]      # [h]
        gbias = H['gat_bias'][li]     # [h]

        # ---- build P table rows
        if li == 0:
            for t in range(10):
                ps = k.psum(Mb)
                wo = H['wp_gat'].index[f'g0.0'][0]
                nc.tensor.matmul(ps[:, :], nf0[:, t * 128:(t + 1) * 128],
                                 wg[:128, wo:wo + Mb], start=True, stop=True)
                pt = gact.tile([128, Mb], FP32, tag="ptile")
                nc.vector.tensor_copy(pt[:], ps[:, :Mb])
                nc.sync.dma_start(table[t * 128:(t + 1) * 128, 0:Mb], pt[:])
        else:
            F_prev = GAT_CFG[li - 1][2] * GAT_CFG[li - 1][4]
            ktp = tiles_of(inn)
            assert inn == F_prev
            for g in range(4):
                for rc in range(3):
                    vs = 128 if rc < 2 else 64
                    nft = gact.tile([128, inn], FP32, tag="nft")
                    nc.sync.dma_start(nft[:vs], ag_prev[g, rc * 128:rc * 128 + vs, :])
                    ps = k.psum(Mb)
                    for ki, ks in enumerate(ktp):
                        tp = k.psum(128)
                        nc.tensor.transpose(tp[:ks, :vs], nft[:vs, ki * 128:ki * 128 + ks],
                                            eye[:vs, :vs])
                        nfT = gact.tile([128, 128], FP32, tag="nfT")
                        nc.vector.tensor_copy(nfT[:ks, :vs], tp[:ks, :vs])
                        wo = H['wp_gat'].index[f'g{li}.{ki}'][0]
                        nc.tensor.matmul(ps[:vs, :], nfT[:ks, :vs],
                                         wg[:ks, wo:wo + Mb],
                                         start=(ki == 0), stop=(ki == len(ktp) - 1))
                    pt = gact.tile([128, Mb], FP32, tag="ptile")
                    nc.vector.tensor_copy(pt[:vs], ps[:vs, :Mb])
                    nc.sync.dma_start(table[g * 320 + rc * 128: g * 320 + rc * 128 + vs,
                                            0:Mb], pt[:vs])

        # ---- gathers
        SRC = gact.tile([128, NCH * Fg], FP32, tag="SRC")
        DST = gact.tile([128, NCH * Fg], FP32, tag="DST")
        for c in range(NCH):
            nc.gpsimd.indirect_dma_start(
                SRC[:, c * Fg:(c + 1) * Fg], None, table[:, :Fg],
                bass.IndirectOffsetOnAxis(ap=gidx[:, c:c + 1], axis=0))
            nc.gpsimd.indirect_dma_start(
                DST[:, c * Fg:(c + 1) * Fg], None, table[:, :Fg],
                bass.IndirectOffsetOnAxis(ap=gidx[:, NCH + c:NCH + c + 1], axis=0))
        SRCv = SRC.rearrange("p (c f) -> p c f", f=Fg)
        DSTv = DST.rearrange("p (c f) -> p c f", f=Fg)

        # ---- edge features
        if li == 0:
            EF = gact.tile([128, NCH * 3], FP32, tag="EF")
            EFv = EF.rearrange("p (c f) -> p c f", f=3)
            nc.vector.tensor_tensor(out=EFv, in0=SRCv[:, :, TBL_GM:TBL_GM + 3],
                                    in1=DSTv[:, :, TBL_GM:TBL_GM + 3], op=ALU.subtract)
            nc.scalar.activation(EFv, EFv, AF.Abs)
        else:
            EF, EFv = fout_prev, fout_prev.rearrange("p (c f) -> p c f",
                                                     f=GAT_CFG[li - 1][4])

        # ---- fout = Lrelu(P_ni[src] + P_nj[dst] + ef @ wfij.T + b)
        FOUT = gact.tile([128, NCH * h], FP32, tag=f"FOUT{li % 2}")
        FOUTv = FOUT.rearrange("p (c f) -> p c f", f=h)
        TMP = gact.tile([128, NCH], FP32, tag="TMP")
        for j in range(h):
            fj = FOUTv[:, :, j]
            nc.vector.tensor_tensor(out=fj, in0=SRCv[:, :, Mw + j],
                                    in1=DSTv[:, :, Mw + h + j], op=ALU.add)
            for kk in range(3):
                nc.vector.tensor_scalar_mul(TMP[:, :NCH], EFv[:, :, kk],
                                            float(wfij[j, kk]))
                nc.vector.tensor_tensor(out=fj, in0=fj, in1=TMP[:, :NCH], op=ALU.add)
            nc.vector.tensor_scalar_add(fj, fj, float(gbias[j]))
            nc.vector.tensor_scalar_mul(TMP[:, :NCH], fj, 0.2)
            nc.vector.tensor_tensor(out=fj, in0=fj, in1=TMP[:, :NCH], op=ALU.max)

        # ---- ex = exp(fout * attn)  (global-shift-free softmax numerator)
        EX = gact.tile([128, NCH * h], FP32, tag="EX")
        EXv = EX.rearrange("p (c f) -> p c f", f=h)
        for j in range(h):
            nc.scalar.activation(EXv[:, :, j], FOUTv[:, :, j], AF.Exp,
                                 scale=float(attn[j]))

        # ---- weighted = [P_node[src] * ex | ex]
        WTD = gact.tile([128, NCH * Fs], FP32, tag="WTD")
        WTDv = WTD.rearrange("p (c f) -> p c f", f=Fs)
        for c in range(NCH):
            for j in range(h):
                nc.vector.tensor_scalar_mul(
                    WTD[:, c * Fs + j * on: c * Fs + (j + 1) * on],
                    SRC[:, c * Fg + j * on: c * Fg + (j + 1) * on],
                    EX[:, c * h + j: c * h + j + 1])
        nc.vector.tensor_copy(WTDv[:, :, Mw:Mw + h], EXv)

        # ---- scatter per node-tile + normalize
        ccg_in = nc.dram_tensor(f"ccg{li}_in", [384, Mw], FP32)
        ccg_out = nc.dram_tensor(f"ccg{li}_out", [4, 384, Mw], FP32)
        for nt in range(3):
            ps = k.psum(Fs)
            for s in range(NSLOT):
                c = nt * NSLOT + s
                nc.tensor.matmul(ps[:, :], St[:, c * 128:(c + 1) * 128],
                                 WTD[:, c * Fs:(c + 1) * Fs],
                                 start=(s == 0), stop=(s == NSLOT - 1))
            NF = gact.tile([128, Mw], FP32, tag="NF")
            DEN = gact.tile([128, h], FP32, tag="DEN")
            nc.vector.tensor_scalar_max(DEN[:], ps[:, Mw:Mw + h], 1e-30)
            nc.vector.reciprocal(DEN[:], DEN[:])
            for j in range(h):
                nc.vector.tensor_scalar_mul(NF[:, j * on:(j + 1) * on],
                                            ps[:, j * on:(j + 1) * on],
                                            DEN[:, j:j + 1])
            nc.sync.dma_start(ccg_in[nt * 128:(nt + 1) * 128, :], NF[:])
        nc.gpsimd.collective_compute("AllGather", ALU.bypass, replica_groups=RG,
                                     ins=[ccg_in[:]], outs=[ccg_out[:]])
        if "gat" in DEBUG:
            dbg_out(k, f"srcg{li}", SRC[:], (128, NCH * Fg))
            dbg_out(k, f"dstg{li}", DST[:], (128, NCH * Fg))
            dbg_out(k, f"ef{li}", EF[:], (128, NCH * (3 if li == 0 else GAT_CFG[li-1][4])))
            ot = nc.dram_tensor(f"dbg_tbl{li}", [N_NODES, TBLW], FP32,
                                kind="ExternalOutput")
            nc.sync.dma_start(ot[:], table[:])
            dbg_out(k, f"fout{li}", FOUT[:], (128, NCH * h))
            dbg_out(k, f"ex{li}", EX[:], (128, NCH * h))
            oag = nc.dram_tensor(f"dbg_agg{li}", [4, 384, Mw], FP32,
                                 kind="ExternalOutput")
            nc.sync.dma_start(oag.rearrange("g n f -> (g n f)"),
                              ccg_out.rearrange("g n f -> (g n f)"))
        ag_prev = ccg_out
        fout_prev = FOUT

    # ---- assemble compact [1280, 32] node output, cls conv
    nfc = nc.dram_tensor("nfc", [N_NODES, 32], FP32)
    for g in range(4):
        for rc in range(3):
            vs = 128 if rc < 2 else 64
            nc.sync.dma_start(nfc[g * 320 + rc * 128: g * 320 + rc * 128 + vs, :],
                              ag_prev[g, rc * 128:rc * 128 + vs, :])
    lastr = gact.tile([128, 1280], FP32, tag="lastr")
    nc.sync.dma_start(lastr[:32], nfc.rearrange("(a b) c -> a (b c)", a=32))
    wo = H['wp_gat'].index['cls'][0]
    bcls = H['gat_cls_b']  # np [128]
    bct = k.pool("consts").tile([128, 1], FP32, tag="bcls")
    nc.sync.dma_start(bct[:], k.io['b_cls'][:])
    last = gact.tile([128, 1280], FP32, tag="last")
    for c0 in range(0, 1280, 512):
        n = min(512, 1280 - c0)
        ps = k.psum(n)
        nc.tensor.matmul(ps[:, :], wg[:32, wo:wo + 128], lastr[:32, c0:c0 + n],
                         start=True, stop=True)
        nc.scalar.activation(last[:, c0:c0 + n], ps[:, :n], AF.Identity, bias=bct[:, 0:1])
    nc.sync.dma_start(k.io['last'][:], last[:])


# ================================================================ build

def build_program(H):
    nc = bacc.Bacc("TRN2", target_bir_lowering=False, debug=False, num_devices=N_CORES)
    io = {}

    def inp(name, shape, dt):
        io[name] = nc.dram_tensor(name, list(shape), dt, kind="ExternalInput")

    def outp(name, shape, dt=FP32):
        io[name] = nc.dram_tensor(name, list(shape), dt, kind="ExternalOutput")

    inp("x27", (27, 71 * 320), CONV_DT)
    inp("masks", (128, 8), FP32)
    inp("w_stem", (128, H['wp_stem'].len), CONV_DT)
    inp("b_stem", (128, H['bp_stem'].len), FP32)
    for sn in ("s1", "s2", "s3", "s4"):
        inp(f"w_{sn}", (128, H[f'wlen_{sn}'] * H[f'nb_{sn}']), CONV_DT)
        inp(f"b_{sn}", (128, H[f'bp_{sn}'].len), FP32)
        inp(f"hidx_{sn}", (128, 2 * H[f'ktm_{sn}']), INT32)
    inp("w_red", (128, H['wp_red'].len), CONV_DT)
    inp("w_gat", (128, H['wp_gat'].len), FP32)
    inp("b_cls", (128, 1), FP32)
    inp("S", (128, 3 * H['NSLOT'] * 128), FP32)
    inp("gidx", (128, 2 * 3 * H['NSLOT']), INT32)
    inp("gm", (N_NODES, 3), FP32)
    inp("eye", (128, 128), FP32)

    outp("feats0", (128, 32 * 160))
    outp("feats1", (128, 2 * 16 * 80))
    outp("feats2", (128, 4 * 8 * 40))
    outp("feats3", (128, 8 * 8 * 40))
    outp("last", (128, 1280))

    order = ["stem", "s1", "s2", "s3", "s4", "red", "gat", "all"]
    lim = order.index(UPTO)
    done = ["feats0"]

    with ExitStack() as ctx:
        tc = ctx.enter_context(tile.TileContext(nc))
        k = K(nc, tc, ctx, H, io)
        for pn, bf, sp in (("consts", 1, "SBUF"), ("psum", 8, "PSUM"),
                           ("dbgp", 2, "SBUF")):
            k.pool(pn, bufs=bf, space=sp)
        pooled = emit_stem(k)
        while lim > 0:
            s1 = [dict(Cin=128 if b == 0 else 256, Cm=64, Cout=256, d=1, W=80,
                       rows=16, inW=80, inRows=16, stride=1, first=(b == 0))
                  for b in range(3)]
            s2 = [dict(Cin=256 if b == 0 else 512, Cm=128, Cout=512, d=1,
                       W=40, rows=8, inW=80 if b == 0 else 40,
                       inRows=16 if b == 0 else 8, stride=2 if b == 0 else 1,
                       first=(b == 0)) for b in range(4)]
            s3 = [dict(Cin=512 if b == 0 else 1024, Cm=256, Cout=1024, d=2, W=40,
                       rows=8, inW=40, inRows=8, stride=1, first=(b == 0))
                  for b in range(23)]
            s4 = [dict(Cin=1024 if b == 0 else 2048, Cm=512, Cout=2048, d=4, W=40,
                       rows=8, inW=40, inRows=8, stride=1, first=(b == 0))
                  for b in range(3)]
            y = emit_stage(k, "s1", s1, pooled, "feats1")  # y is carry name
            done.append("feats1")
            if lim <= order.index("s1"):
                break
            y = emit_stage(k, "s2", s2, y, "feats2")
            done.append("feats2")
            if lim <= order.index("s2"):
                break
            y = emit_stage(k, "s3", s3, y, "feats3")
            done.append("feats3")
            if lim <= order.index("s3"):
                break
            y = emit_stage(k, "s4", s4, y)
            if lim <= order.index("s4"):
                break
            # red conv -> nf slice -> AG -> nf0 [128, 1280]
            wr = load_plane(k, "gat_w", "w_red", CONV_DT)
            y4 = k.pool("gat", bufs=1).tile([128, 16 * 320], CONV_DT, tag="y4",
                                            name="y4")
            nc.sync.dma_start(y4[:], k.io[y][:])
            nf_my = k.pool("gat", bufs=1).tile([128, 320], FP32, tag="nf_my",
                                               name="nf_my")
            wpr = H['wp_red']
            ps = k.psum(320)
            for ki in range(16):
                wo, wk, wm = wpr.index[f'red.{ki}.0']
                nc.tensor.matmul(ps[:, :], wr[:wk, wo:wo + wm],
                                 y4[:128, ki * 320:(ki + 1) * 320],
                                 start=(ki == 0), stop=(ki == 15))
            nc.vector.tensor_copy(nf_my[:], ps[:, :320])
            ccr_in = nc.dram_tensor("ccr_in", [128, 320], FP32)
            ccr_out = nc.dram_tensor("ccr_out", [4, 128, 320], FP32)
            nc.sync.dma_start(ccr_in[:], nf_my[:])
            nc.gpsimd.collective_compute("AllGather", ALU.bypass, replica_groups=RG,
                                         ins=[ccr_in[:]], outs=[ccr_out[:]])
            nf0 = k.pool("gat", bufs=1).tile([128, 1280], FP32, tag="nf0")
            nc.sync.dma_start(nf0[:], ccr_out.rearrange("g p f -> p g f"))
            if "nf0" in DEBUG:
                dbg_out(k, "nf0", nf0[:], (128, 1280))
            if lim <= order.index("red"):
                break
            emit_gat(k, nf0)
            done.append("last")
            break
        _finish_dummy(k, [n for n in ("feats1", "feats2", "feats3", "last")
                          if n not in done])

    nc.finalize()
    return nc


def _finish_dummy(k, names):
    """write zeros to unused outputs so every declared output is produced"""
    nc = k.nc
    for n in names:
        shape = list(k.io[n].shape)
        t = k.pool("dbgp", bufs=2).tile(shape, FP32, tag="dbg")
        nc.gpsimd.memset(t[:], 0.0)
        nc.sync.dma_start(k.io[n][:], t[:])


# ================================================================ host side

def prep_host(params, x, final_x, src, dst):
    """Build all host-side constants. Returns H (shared) and per-core input maps."""
    H = {}
    rp = params['resnet']

    # ---- stem pack
    wp, bp = Plane(CONV_DT), Plane(FP32)
    w1, b1 = fold_bn(rp['c1'], rp['b1'])
    wp.add('c1', w1.transpose(2, 3, 1, 0).reshape(27, 64))
    bp_add(bp, 'c1', b1, [64])
    w2, b2 = fold_bn(rp['c2'], rp['b2'])
    for dy in range(3):
        for dx in range(3):
            wp.add(f'c2.{dy}{dx}', w2[:, :, dy, dx].T)
    bp_add(bp, 'c2', b2, [64])
    w3, b3 = fold_bn(rp['c3'], rp['b3'])
    for dy in range(3):
        for dx in range(3):
            wp.add(f'c3.{dy}{dx}', w3[:, :, dy, dx].T)
    bp_add(bp, 'c3', b3, [128])
    H['wp_stem'], H['bp_stem'] = wp, bp

    # ---- stage packs (per-block planes concatenated)
    for si, (planes, nb, stride, dil) in enumerate(STAGES):
        sname = f"s{si + 1}"
        bp = Plane(FP32)
        block_planes = []
        wlen = None
        for bi, blk in enumerate(rp['layers'][si]):
            wpb = Plane(CONV_DT)
            name = f"{sname}b{bi}"
            wc1, bc1 = fold_bn(blk['w1'], blk['bn1'])
            pack_1x1(wpb, bp, f"{name}.c1", wc1[:, :, 0, 0], bc1)
            wc2, bc2 = fold_bn(blk['w2'], blk['bn2'])
            pack_3x3(wpb, bp, f"{name}.c2", wc2, bc2)
            wc3, bc3 = fold_bn(blk['w3'], blk['bn3'])
            pack_1x1(wpb, bp, f"{name}.c3", wc3[:, :, 0, 0], bc3)
            if 'dw' in blk:
                wd, bd = fold_bn(blk['dw'], blk['dbn'])
                pack_1x1(wpb, bp, f"{name}.ds", wd[:, :, 0, 0], bd)
            block_planes.append(wpb)
        wlen = max(p.len for p in block_planes)
        # pad all blocks to wlen and merge indexes with block offsets
        wp = Plane(CONV_DT)
        planes_np = []
        for bi, p in enumerate(block_planes):
            arr = p.plane()
            if arr.shape[1] < wlen:
                arr = np.concatenate(
                    [arr, np.zeros((128, wlen - arr.shape[1]), arr.dtype)], axis=1)
            planes_np.append(arr)
            for nm, (off, Kd, M) in p.index.items():
                wp.index[nm] = (off, Kd, M)   # offsets are block-relative
            for nm, v in p.gindex.items():
                wp.gindex[nm] = v
        H[f'wp_{sname}'] = wp
        H[f'bp_{sname}'] = bp
        H[f'wlen_{sname}'] = wlen
        H[f'nb_{sname}'] = nb
        H[f'ktm_{sname}'] = len(tiles_of(planes))
        H[f'wplane_{sname}'] = np.concatenate(planes_np, axis=1)

    # ---- red conv
    wp = Plane(CONV_DT)
    wred = _np(params['red'])[:, :, 0, 0]   # [128, 2048]
    pack_1x1(wp, Plane(FP32), 'red', wred, None)
    H['wp_red'] = wp

    # ---- GAT + cls pack (fp32)
    wp = Plane(FP32)
    gp = params['gat']
    H['gat_wfij'], H['gat_attn'], H['gat_bias'] = [], [], []
    for li, (inn, ine, on, oe, h) in enumerate(GAT_CFG):
        g = gp[li]
        WbT = np.concatenate([_np(g['w_node']).T, _np(g['w_ni']).T,
                              _np(g['w_nj']).T], axis=1)  # [inn, Mb]
        for ki, ks in enumerate(tiles_of(inn)):
            wp.add(f'g{li}.{ki}', WbT[ki * 128:ki * 128 + ks, :])
        H['gat_wfij'].append(_np(g['w_fij']).astype(np.float32))
        H['gat_attn'].append(_np(g['attn']).reshape(h).astype(np.float32))
        H['gat_bias'].append(_np(g['bias']).reshape(h).astype(np.float32))
    wp.add('cls', _np(params['cls_w'])[:, :, 0, 0].T)   # [32, 128]
    H['wp_gat'] = wp
    bcls = np.zeros((128, 1), np.float32)
    bcls[:, 0] = _np(params['cls_b'])
    H['gat_cls_b'] = bcls

    # ---- graph structures
    src_n, dst_n = _np(src).astype(np.int64), _np(dst).astype(np.int64)
    order = np.argsort(dst_n, kind='stable')
    NT = 10  # global node tiles (128 each)
    # per (quarter, local tile 0..2): edges
    edges_by_tile = [[[] for _ in range(3)] for _ in range(4)]
    for e in order:
        d_ = dst_n[e]
        q, loc = divmod(d_, 320)
        edges_by_tile[q][loc // 128].append(e)
    NSLOT = max(1, max((len(v) + 127) // 128
                       for qv in edges_by_tile for v in qv))
    H['NSLOT'] = NSLOT
    NCH = 3 * NSLOT
    EPQ = NCH * 128
    # per quarter arrays
    gidx_q, S_q = [], []
    for q in range(4):
        gi = np.zeros((128, 2 * NCH), np.int32)
        S = np.zeros((3, NSLOT, 128, 128), np.float32)
        for nt in range(3):
            ev = edges_by_tile[q][nt]
            for i, e in enumerate(ev):
                s_, p_ = divmod(i, 128)
                c = nt * NSLOT + s_
                gi[p_, c] = src_n[e]
                gi[p_, NCH + c] = dst_n[e]
                S[nt, s_, p_, dst_n[e] - 320 * q - nt * 128] = 1.0
        gidx_q.append(gi)
        # S device layout: [128 edges, (nt, slot, node)]
        S_q.append(S.transpose(2, 0, 1, 3).reshape(128, 3 * NSLOT * 128))

    # ---- per-core inputs
    xf = _np(x).astype(np.float32)
    gmf = _np(final_x).astype(np.float32)
    B = xf.shape[0]
    in_maps = []
    shared = {
        'w_stem': H['wp_stem'].plane(), 'b_stem': H['bp_stem'].plane(),
        'w_red': H['wp_red'].plane(), 'w_gat': H['wp_gat'].plane(),
        'b_cls': H['gat_cls_b'], 'eye': np.eye(128, dtype=np.float32),
    }
    for s in ('s1', 's2', 's3', 's4'):
        shared[f'w_{s}'] = H[f'wplane_{s}']
        shared[f'b_{s}'] = H[f'bp_{s}'].plane()

    for core in range(N_CORES):
        g, q = divmod(core, 4)
        m = dict(shared)
        # x27 im2col
        x27 = np.zeros((27, 71, 320), np_dt(CONV_DT))
        xg = xf[g]
        for dy in range(3):
            for dx in range(3):
                for c in range(3):
                    p = (dy * 3 + dx) * 3 + c
                    for r in range(71):
                        gr = 64 * q - 3 + r + 2 * (dy - 1)
                        if 0 <= gr < 256:
                            cl = 2 * (dx - 1)
                            lo, hi = max(0, -cl), min(320, 320 - cl)
                            x27[p, r, lo:hi] = xg[c, gr, lo + cl:hi + cl]
        m['x27'] = x27.reshape(27, 71 * 320)
        # masks
        mk = np.ones((128, 8), np.float32)
        if q == 0:
            mk[:, 0] = 0; mk[:, 1] = 0
        if q == 3:
            mk[:, 2] = 0; mk[:, 3] = 0; mk[:, 4] = 0
        m['masks'] = mk
        # halo gather indices per stage
        for si, (planes, nb, stride, dil) in enumerate(STAGES):
            sname = f"s{si + 1}"
            Cm = planes
            KT = len(tiles_of(Cm))
            hi = np.full((128, 2 * KT), SENT, np.int32)
            for kt, ks in enumerate(tiles_of(Cm)):
                for p in range(ks):
                    ch = kt * 128 + p
                    if q > 0:
                        hi[p, 2 * kt] = (q - 1) * Cm * 2 + ch * 2 + 1
                    if q < 3:
                        hi[p, 2 * kt + 1] = (q + 1) * Cm * 2 + ch * 2 + 0
            m[f'hidx_{sname}'] = hi
        m['gidx'] = gidx_q[q]
        m['S'] = S_q[q]
        m['gm'] = gmf[g]
        in_maps.append(m)
    return H, in_maps


def kernel(params, x, final_x, src, dst):
    H, in_maps = prep_host(params, x, final_x, src, dst)
    nc = build_program(H)
    res = run_bass_kernel_spmd(nc, in_maps, list(range(N_CORES)))
    return assemble(res, H)


def assemble(res, H):
    B = 2
    f0 = np.zeros((B, 128, 128, 160), np.float32)
    f1 = np.zeros((B, 256, 64, 80), np.float32)
    f2 = np.zeros((B, 512, 32, 40), np.float32)
    f3 = np.zeros((B, 1024, 32, 40), np.float32)
    last = np.zeros((B, 128, 32, 40), np.float32)
    for core in range(N_CORES):
        g, q = divmod(core, 4)
        r = res.results[core]
        f0[g, :, 32 * q:32 * q + 32, :] = r['feats0'].reshape(128, 32, 160)
        a = r['feats1'].reshape(128, 2, 16, 80)
        for kt in range(2):
            f1[g, kt * 128:(kt + 1) * 128, 16 * q:16 * q + 16, :] = a[:, kt]
        a = r['feats2'].reshape(128, 4, 8, 40)
        for kt in range(4):
            f2[g, kt * 128:(kt + 1) * 128, 8 * q:8 * q + 8, :] = a[:, kt]
        a = r['feats3'].reshape(128, 8, 8, 40)
        for kt in range(8):
            f3[g, kt * 128:(kt + 1) * 128, 8 * q:8 * q + 8, :] = a[:, kt]
        if q == 0:
            last[g] = r['last'].reshape(128, 32, 40)
    return f0, f1, f2, f3, last


# revision 26
# speedup vs baseline: 1.6419x; 1.6419x over previous
"""Trainium2 Bass kernel for nn_GAT_edge: dilated ResNet-101 + 3-layer edge-GAT.

Parallelization: 2 images x 4-way spatial split over H (8 NeuronCores).
Halo exchange per bottleneck block via AllGather + indirect-DMA gathers.
GAT: edges partitioned by destination-node quarter; per-layer node-feature
AllGather. Final outputs: per-core row slices, reassembled on host.

kernel(**inputs) -> tuple of 5 np.ndarrays matching reference.py.
"""
import numpy as np
import ml_dtypes
from contextlib import ExitStack

import concourse.bass as bass
import concourse.tile as tile
from concourse import bacc, mybir
from concourse.bass_utils import run_bass_kernel_spmd

FP32 = mybir.dt.float32
BF16 = mybir.dt.bfloat16
INT32 = mybir.dt.int32
AF = mybir.ActivationFunctionType
ALU = mybir.AluOpType

CONV_DT = BF16          # BF16 | FP32 | mybir.dt.float32r
EPS = 1e-5
N_CORES = 8
RG = [[0, 1, 2, 3], [4, 5, 6, 7]]
N_NODES = 1280
STAGES = [(64, 3, 1, 1), (128, 4, 2, 1), (256, 23, 1, 2), (512, 3, 1, 4)]
GAT_CFG = [(128, 3, 64, 1, 3), (192, 3, 32, 1, 3), (96, 3, 32, 1, 1)]
TBLW = 204
TBL_GM = 200
SENT = 1 << 20          # OOB sentinel for halo gathers at image edges

DEBUG = set()           # e.g. {"stem", "s1", "s2", "s3", "s4", "nf0"}
UPTO = "all"            # stem|s1|s2|s3|s4|red|gat|all


def np_dt(dt):
    return ml_dtypes.bfloat16 if dt == BF16 else np.float32


def _np(x):
    return np.asarray(x)


# ================================================================ host packing

class Plane:
    """[128, LEN] plane; blocks at (cols off:off+M, partitions 0:K)."""

    def __init__(self, dt):
        self.cols, self.len, self.index, self.dt = [], 0, {}, dt
        self.gindex = {}

    def add(self, name, blkKM):
        Kd, M = blkKM.shape
        assert Kd <= 128, (name, blkKM.shape)
        blk = np.zeros((128, M), dtype=np_dt(self.dt))
        blk[:Kd] = blkKM.astype(np_dt(self.dt))
        off = self.len
        self.cols.append(blk)
        self.len += M
        self.index[name] = (off, Kd, M)
        return off

    def plane(self):
        if not self.cols:
            return np.zeros((128, 1), dtype=np_dt(self.dt))
        return np.concatenate(self.cols, axis=1)


def tiles_of(C):
    return [min(128, C - c) for c in range(0, C, 128)]


def fold_bn(w, bn):
    g, b, m, v = (_np(bn[x]).astype(np.float64) for x in ('g', 'b', 'm', 'v'))
    s = g / np.sqrt(v + EPS)
    return (_np(w).astype(np.float64) * s[:, None, None, None]).astype(np.float32), \
        (b - m * s).astype(np.float32)


def pack_1x1(wp, bp, name, w, bias):
    O, I = w.shape[:2]
    w = w.reshape(O, I)
    kts, mts = tiles_of(I), tiles_of(O)
    for mi in range(len(mts)):
        g0 = wp.len
        for ki in range(len(kts)):
            wp.add(f"{name}.{ki}.{mi}",
                   w[mi * 128:mi * 128 + mts[mi], ki * 128:ki * 128 + kts[ki]].T)
        wp.gindex[(name, mi)] = (g0, wp.len - g0)
    if bias is not None:
        bp_add(bp, name, bias, mts)
    return dict(kts=kts, mts=mts)


def pack_3x3(wp, bp, name, w, bias):
    O, I = w.shape[:2]
    kts, mts = tiles_of(I), tiles_of(O)
    for mi in range(len(mts)):
        g0 = wp.len
        for dy in range(3):
            for dx in range(3):
                for ki in range(len(kts)):
                    wp.add(f"{name}.{dy}{dx}.{ki}.{mi}",
                           w[mi * 128:mi * 128 + mts[mi],
                             ki * 128:ki * 128 + kts[ki], dy, dx].T)
        wp.gindex[(name, mi)] = (g0, wp.len - g0)
    bp_add(bp, name, bias, mts)
    return dict(kts=kts, mts=mts)


def bp_add(bp, name, bias, mts):
    cols = []
    for mi, ms in enumerate(mts):
        col = np.zeros((128, 1), np.float32)
        col[:ms, 0] = bias[mi * 128:mi * 128 + ms]
        cols.append(col)
    blk = np.concatenate(cols, axis=1)
    off = bp.len
    bp.cols.append(blk)
    bp.len += blk.shape[1]
    bp.index[name] = (off, 128, blk.shape[1])


# ================================================================ device utils

def v3(t, P, base, R, Wp):
    """view [P, R, Wp] of flat tile t at col offset base"""
    return t[:P, base:base + R * Wp].rearrange("p (r w) -> p r w", w=Wp)


class K:
    def __init__(self, nc, tc, ctx, H, io):
        self.nc, self.tc, self.ctx, self.H, self.io = nc, tc, ctx, H, io
        self.pools = {}
        self.gblk = 0

    def pool(self, name, bufs=1, space="SBUF"):
        if name not in self.pools:
            self.pools[name] = self.ctx.enter_context(
                self.tc.tile_pool(name=name, bufs=bufs, space=space))
        return self.pools[name]

    def psum(self, n, tag="ps"):
        return self.pool("psum", bufs=8, space="PSUM").tile(
            [128, n], FP32, tag="ps", name="pst")


def load_plane(k, pool_name, io_name, dt, tag=None, bufs=1):
    t = k.pool(pool_name, bufs=bufs).tile(list(k.io[io_name].shape), dt,
                                          tag=tag or io_name)
    k.nc.sync.dma_start(t[:], k.io[io_name][:])
    return t


def dbg_out(k, name, src_ap, shape):
    """declare debug output and write src (cast to fp32)"""
    nc = k.nc
    o = nc.dram_tensor(f"dbg_{name}", list(shape), FP32, kind="ExternalOutput")
    tmp = k.pool("dbgp", bufs=2).tile(list(shape), FP32, tag="dbg")
    nc.vector.tensor_copy(tmp[:src_ap.shape[0]], src_ap)
    nc.sync.dma_start(o[:src_ap.shape[0]], tmp[:src_ap.shape[0]])


# ================================================================ stem

def emit_stem(k):
    nc, H, io = k.nc, k.H, k.io
    wp, bp = H['wp_stem'], H['bp_stem']
    mk = load_plane(k, "consts", "masks", FP32)
    cd = carry_dram(k, "carry_stem", 16 * 80)

    with ExitStack() as sctx:
        act = sctx.enter_context(k.tc.tile_pool(name="stem_act", bufs=1))
        wpool = sctx.enter_context(k.tc.tile_pool(name="stem_w", bufs=1))
        xpool = sctx.enter_context(k.tc.tile_pool(name="stem_x", bufs=3))
        wt = wpool.tile([128, wp.len], CONV_DT, tag="w_stem", name="w_stem")
        nc.sync.dma_start(wt[:], io['w_stem'][:])
        bt = wpool.tile([128, bp.len], FP32, tag="b_stem", name="b_stem")
        nc.sync.dma_start(bt[:], io['b_stem'][:])

        R1, W1, Wp1 = 71, 320, 322
        c1 = act.tile([128, R1 * Wp1], CONV_DT, tag="c1buf", name="c1buf")
        nc.gpsimd.memset(c1[:64], 0.0)
        woff = wp.index['c1'][0]
        boff = bp.index['c1'][0]
        c1v = v3(c1, 64, 0, R1, Wp1)
        CH = 4
        for r0 in range(0, R1, CH):
            nr = min(CH, R1 - r0)
            xt = xpool.tile([128, CH * W1], CONV_DT, tag="x27c", name="x27c")
            nc.sync.dma_start(xt[:27, :nr * W1],
                              io['x27'][:, r0 * W1:(r0 + nr) * W1])
            for r in range(nr):
                ps = k.psum(W1)
                nc.tensor.matmul(ps[:64], wt[:27, woff:woff + 64],
                                 xt[:27, r * W1:(r + 1) * W1], start=True, stop=True)
                nc.scalar.activation(c1v[:, r0 + r, 1:321], ps[:64], AF.Relu,
                                     bias=bt[:64, boff:boff + 1])
        nc.vector.tensor_scalar_mul(c1v[:, 2, :], c1v[:, 2, :], mk[:64, 0:1])

        # conv2 64->64 k3 s2: out slot i reads c1 slots 2i+dy, col 2c+dx
        R2, W2, Wp2 = 35, 160, 162
        c2 = act.tile([128, R2 * Wp2], CONV_DT, tag="c2buf", name="c2buf")
        nc.gpsimd.memset(c2[:64], 0.0)
        boff2 = bp.index['c2'][0]
        c2v = v3(c2, 64, 0, R2, Wp2)
        for r0 in range(0, R2, 3):
            nr = min(3, R2 - r0)
            ps = k.psum(nr * W2)
            acc = 0
            for dy in range(3):
                for dx in range(3):
                    wo = wp.index[f'c2.{dy}{dx}'][0]
                    rhs = c1v[:, 2 * r0 + dy: 2 * r0 + dy + 2 * nr - 1:2,
                              dx:dx + 2 * W2 - 1:2]
                    nc.tensor.matmul(ps[:64], wt[:64, wo:wo + 64], rhs,
                                     start=(acc == 0), stop=(acc == 8))
                    acc += 1
            nc.scalar.activation(c2v[:, r0:r0 + nr, 1:161],
                                 ps[:64].rearrange("p (r w) -> p r w", r=nr),
                                 AF.Relu, bias=bt[:64, boff2:boff2 + 1])
        nc.vector.tensor_scalar_mul(c2v[:, 0, :], c2v[:, 0, :], mk[:64, 1:2])
        nc.vector.tensor_scalar_mul(c2v[:, 33, :], c2v[:, 33, :], mk[:64, 2:3])
        nc.vector.tensor_scalar_mul(c2v[:, 34, :], c2v[:, 34, :], mk[:64, 3:4])

        # conv3 64->128 k3 p1: out slot i reads c2 slots i+dy, col c+dx
        R3, W3, Wp3 = 33, 160, 162
        c3 = act.tile([128, R3 * Wp3], CONV_DT, tag="c3buf", name="c3buf")
        nc.gpsimd.memset(c3[:], 0.0)
        boff3 = bp.index['c3'][0]
        c3v = v3(c3, 128, 0, R3, Wp3)
        for r0 in range(0, R3, 3):
            nr = min(3, R3 - r0)
            ps = k.psum(nr * W3)
            acc = 0
            for dy in range(3):
                for dx in range(3):
                    wo = wp.index[f'c3.{dy}{dx}'][0]
                    rhs = c2v[:, r0 + dy: r0 + dy + nr, dx:dx + W3]
                    nc.tensor.matmul(ps[:, :], wt[:64, wo:wo + 128], rhs,
                                     start=(acc == 0), stop=(acc == 8))
                    acc += 1
            nc.scalar.activation(c3v[:, r0:r0 + nr, 1:161],
                                 ps.rearrange("p (r w) -> p r w", r=nr),
                                 AF.Relu, bias=bt[:, boff3:boff3 + 1])
        nc.vector.tensor_scalar_mul(c3v[:, 32, :], c3v[:, 32, :], mk[:, 4:5])

        # feats0 = c3 rows [0:32)
        if CONV_DT == FP32:
            nc.sync.dma_start(io['feats0'].rearrange("p (r w) -> p r w", w=160),
                              c3v[:, 0:32, 1:161])
        else:
            f0 = act.tile([128, 32 * 160], FP32, tag="f0", name="f0")
            nc.vector.tensor_copy(f0.rearrange("p (r w) -> p r w", r=32),
                                  c3v[:, 0:32, 1:161])
            nc.sync.dma_start(io['feats0'][:], f0[:])

        # maxpool k3 s2 (ceil)
        pooled = act.tile([128, 16 * 80], CONV_DT, tag="pooled", name="pooled")
        pv = pooled.rearrange("p (r w) -> p r w", r=16)
        first = True
        for dy in range(3):
            for dx in range(3):
                src = c3v[:, dy:dy + 31:2, 1 + dx:1 + dx + 159:2]
                if first:
                    nc.vector.tensor_copy(pv, src)
                    first = False
                else:
                    nc.vector.tensor_tensor(out=pv, in0=pv, in1=src, op=ALU.max)
        nc.sync.dma_start(cd[:], pooled[:])
        if "stem" in DEBUG:
            dbg_out(k, "pooled", pooled[:], (128, 16 * 80))
    return "carry_stem"


def emit_block(k, sname, bi, geo, x, y, wpool, bt, cc_in, cc_out, hidx, resources):
    """One bottleneck block with streamed per-(conv, m-tile) weights."""
    nc, H = k.nc, k.H
    wp, bp = H[f'wp_{sname}'], H[f'bp_{sname}']
    wdram = k.io[f'w_{sname}']
    wbase = bi * H[f'wlen_{sname}']
    Cin, Cm, Cout = geo['Cin'], geo['Cm'], geo['Cout']
    d, W, rows = geo['d'], geo['W'], geo['rows']
    inW, inRows, stride = geo['inW'], geo['inRows'], geo['stride']
    first = geo['first']
    kti, ktm, kto = tiles_of(Cin), tiles_of(Cm), tiles_of(Cout)
    name = f"{sname}b{bi}"
    pb, c2o, resid = resources['pb'], resources['c2o'], resources['resid']

    PR, PW = inRows + 2 * d, inW + 2 * d
    npx_in = inRows * inW
    npx = rows * W

    def wload(cname, mi, tag):
        off, ln = wp.gindex[(f'{name}.{cname}', mi)]
        t = wpool.tile([128, ln], CONV_DT, tag=tag, name=f"w_{tag}")
        nc.sync.dma_start(t[:], wdram[:, wbase + off: wbase + off + ln])
        return t

    # --- conv1 (1x1 at input res) -> pb interior
    bo1 = bp.index[f'{name}.c1'][0]
    chunk = max(1, 512 // inW) * inW
    for mi, ms in enumerate(ktm):
        wt1 = wload('c1', mi, 'wc1')
        c0 = 0
        while c0 < npx_in:
            n = min(chunk, npx_in - c0)
            nr, r0 = n // inW, c0 // inW
            ps = k.psum(n)
            for ki, ks in enumerate(kti):
                nc.tensor.matmul(ps[:ms], wt1[:ks, ki * ms:(ki + 1) * ms],
                                 x[:ks, ki * npx_in + c0: ki * npx_in + c0 + n],
                                 start=(ki == 0), stop=(ki == len(kti) - 1))
            pbv = v3(pb, ms, mi * PR * PW, PR, PW)
            nc.scalar.activation(pbv[:, d + r0: d + r0 + nr, d:d + inW],
                                 ps[:ms].rearrange("p (r w) -> p r w", r=nr),
                                 AF.Relu, bias=bt[:ms, bo1 + mi:bo1 + mi + 1])
            c0 += n

    # --- halo exchange on conv1 output
    for mi, ms in enumerate(ktm):
        pbv = v3(pb, ms, mi * PR * PW, PR, PW)
        nc.sync.dma_start(cc_in[mi * 128:mi * 128 + ms, 0], pbv[:, d:2 * d, d:d + inW])
        nc.sync.dma_start(cc_in[mi * 128:mi * 128 + ms, 1],
                          pbv[:, inRows:inRows + d, d:d + inW])
    nc.gpsimd.collective_compute("AllGather", ALU.bypass, replica_groups=RG,
                                 ins=[cc_in[:]], outs=[cc_out[:]])
    ccf = cc_out.rearrange("g c s d w -> (g c s) (d w)")
    nunits = 4 * Cm * 2
    hsc = resources['hsc']
    hw = d * inW
    for mi, ms in enumerate(ktm):
        pbv = v3(pb, ms, mi * PR * PW, PR, PW)
        sc_t = hsc[:ms, (2 * mi) * hw:(2 * mi + 1) * hw]
        sc_b = hsc[:ms, (2 * mi + 1) * hw:(2 * mi + 2) * hw]
        nc.gpsimd.indirect_dma_start(
            sc_t, None, ccf[:],
            bass.IndirectOffsetOnAxis(ap=hidx[:ms, 2 * mi:2 * mi + 1], axis=0),
            bounds_check=nunits - 1, oob_is_err=False)
        nc.gpsimd.indirect_dma_start(
            sc_b, None, ccf[:],
            bass.IndirectOffsetOnAxis(ap=hidx[:ms, 2 * mi + 1:2 * mi + 2], axis=0),
            bounds_check=nunits - 1, oob_is_err=False)
        nc.vector.tensor_copy(pbv[:, 0:d, d:d + inW],
                              sc_t.rearrange("p (d w) -> p d w", d=d))
        nc.vector.tensor_copy(pbv[:, inRows + d:inRows + 2 * d, d:d + inW],
                              sc_b.rearrange("p (d w) -> p d w", d=d))

    # --- conv2 (3x3 dil d, stride)
    bo2 = bp.index[f'{name}.c2'][0]
    nkt = len(ktm)
    crows = max(1, 512 // W)
    for mi, ms in enumerate(ktm):
        wt2 = wload('c2', mi, 'wc2')
        for r0 in range(0, rows, crows):
            nr = min(crows, rows - r0)
            ps = k.psum(nr * W)
            acc, nacc = 0, 9 * nkt
            for dy in range(3):
                for dx in range(3):
                    rb, cb = dy * d + r0 * stride, dx * d
                    for ki, ks in enumerate(ktm):
                        lo = ((dy * 3 + dx) * nkt + ki) * ms
                        pbv = v3(pb, ks, ki * PR * PW, PR, PW)
                        rhs = pbv[:, rb:rb + (nr - 1) * stride + 1:stride,
                                  cb:cb + (W - 1) * stride + 1:stride]
                        nc.tensor.matmul(ps[:ms], wt2[:ks, lo:lo + ms], rhs,
                                         start=(acc == 0), stop=(acc == nacc - 1))
                        acc += 1
            nc.scalar.activation(c2o[:ms, mi * npx + r0 * W: mi * npx + (r0 + nr) * W],
                                 ps[:ms], AF.Relu, bias=bt[:ms, bo2 + mi:bo2 + mi + 1])

    # --- downsample residual (block 1)
    if first:
        bod = bp.index[f'{name}.ds'][0]
        crows = max(1, 512 // W)
        for mi, ms in enumerate(kto):
            wtd = wload('ds', mi, 'wcd')
            for r0 in range(0, rows, crows):
                nr = min(crows, rows - r0)
                ps = k.psum(nr * W)
                for ki, ks in enumerate(kti):
                    if stride == 1:
                        rhs = x[:ks, ki * npx_in + r0 * W: ki * npx_in + (r0 + nr) * W]
                    else:
                        xv = v3(x, ks, ki * npx_in, inRows, inW)
                        rhs = xv[:, r0 * stride:(r0 + nr - 1) * stride + 1:stride,
                                 0:inW - stride + 1:stride]
                    nc.tensor.matmul(ps[:ms], wtd[:ks, ki * ms:(ki + 1) * ms], rhs,
                                     start=(ki == 0), stop=(ki == len(kti) - 1))
                nc.scalar.activation(
                    resid[:ms, mi * npx + r0 * W: mi * npx + (r0 + nr) * W],
                    ps[:ms], AF.Identity, bias=bt[:ms, bod + mi:bod + mi + 1])
        res_t, res_F = resid, npx
    else:
        res_t, res_F = x, npx_in

    # --- conv3 (1x1) + residual + relu -> y
    bo3 = bp.index[f'{name}.c3'][0]
    for mi, ms in enumerate(kto):
        wt3 = wload('c3', mi, 'wc3')
        for c0 in range(0, npx, 512):
            n = min(512, npx - c0)
            ps = k.psum(n)
            for ki, ks in enumerate(ktm):
                nc.tensor.matmul(ps[:ms], wt3[:ks, ki * ms:(ki + 1) * ms],
                                 c2o[:ks, ki * npx + c0: ki * npx + c0 + n],
                                 start=(ki == 0), stop=(ki == len(ktm) - 1))
            ys = y[:ms, mi * npx + c0: mi * npx + c0 + n]
            nc.vector.tensor_tensor(out=ys, in0=ps[:ms],
                                    in1=res_t[:ms, mi * res_F + c0: mi * res_F + c0 + n],
                                    op=ALU.add)
            nc.scalar.activation(ys, ys, AF.Relu, bias=bt[:ms, bo3 + mi:bo3 + mi + 1])

    if k.H.get('tapblk') == (sname, bi):
        KTm, KTo = len(ktm), len(kto)
        o1 = nc.dram_tensor("dbg_ccin", [Cm * 2 * d * inW], FP32, kind="ExternalOutput")
        nc.sync.dma_start(o1[:], cc_in.rearrange("c s d w -> (c s d w)"))
        o2 = nc.dram_tensor("dbg_ccout", [4 * Cm * 2 * d * inW], FP32,
                            kind="ExternalOutput")
        nc.sync.dma_start(o2[:], cc_out.rearrange("g c s d w -> (g c s d w)"))
        dbg_out(k, "pb", pb[:], (128, KTm * PR * PW))
        dbg_out(k, "c2o", c2o[:], (128, KTm * npx))
        if first:
            dbg_out(k, "resid", resid[:], (128, KTo * npx))
        dbg_out(k, "yblk", y[:], (128, KTo * npx))


def carry_dram(k, name, n):
    if name not in k.io:
        k.io[name] = k.nc.dram_tensor(name, [128, n], CONV_DT)
    return k.io[name]


def emit_stage(k, sname, blocks_geo, carry_in, feats_name=None):
    """Run blocks of a stage in a scoped pool; returns output carry name."""
    nc, H, io = k.nc, k.H, k.io
    gN = blocks_geo[-1]
    Cm = blocks_geo[0]['Cm']
    ktm, kto = tiles_of(Cm), tiles_of(gN['Cout'])
    rows, W = gN['rows'], gN['W']
    npx = rows * W

    with ExitStack() as sctx:
        act = sctx.enter_context(k.tc.tile_pool(name=f"{sname}_act", bufs=1))
        wpool = sctx.enter_context(k.tc.tile_pool(name=f"{sname}_w", bufs=2))

        geoms = {}
        for geo in blocks_geo:
            key = (geo['inRows'], geo['inW'], geo['d'])
            if key not in geoms:
                gi = len(geoms)
                PR, PW = geo['inRows'] + 2 * geo['d'], geo['inW'] + 2 * geo['d']
                pb = act.tile([128, len(ktm) * PR * PW], CONV_DT,
                              tag=f"{sname}_pb{gi}", name=f"pb_{sname}{gi}")
                nc.gpsimd.memset(pb[:], 0.0)
                hscg = act.tile([128, 2 * len(ktm) * geo['d'] * geo['inW']], CONV_DT,
                                tag=f"{sname}_hsc{gi}", name=f"hsc_{sname}{gi}")
                nc.gpsimd.memset(hscg[:], 0.0)
                ccs = []
                for par in range(2):
                    ci = nc.dram_tensor(f"cc_{sname}_{gi}_{par}_in",
                                        [Cm, 2, geo['d'], geo['inW']], CONV_DT)
                    co = nc.dram_tensor(f"cc_{sname}_{gi}_{par}_out",
                                        [4, Cm, 2, geo['d'], geo['inW']], CONV_DT)
                    ccs.append((ci, co))
                geoms[key] = (pb, ccs, hscg)

        c2o = act.tile([128, len(ktm) * npx], CONV_DT, tag=f"{sname}_c2o",
                       name=f"c2o_{sname}")
        resid = act.tile([128, len(kto) * npx], CONV_DT, tag=f"{sname}_res",
                         name=f"res_{sname}")
        hidx = k.pool("consts").tile([128, 2 * len(ktm)], INT32,
                                     tag=f"hidx_{sname}", name=f"hx_{sname}")
        nc.sync.dma_start(hidx[:], io[f'hidx_{sname}'][:])
        bt = k.pool("consts").tile([128, H[f'bp_{sname}'].len], FP32,
                                   tag=f"b_{sname}", name=f"bt_{sname}")
        nc.sync.dma_start(bt[:], io[f'b_{sname}'][:])

        kin = tiles_of(blocks_geo[0]['Cin'])
        npx_in0 = blocks_geo[0]['inRows'] * blocks_geo[0]['inW']
        x = act.tile([128, len(kin) * npx_in0], CONV_DT, tag=f"{sname}_x0",
                     name=f"x0_{sname}")
        nc.sync.dma_start(x[:], k.io[carry_in][:])
        ya = act.tile([128, len(kto) * npx], CONV_DT, tag=f"{sname}_ya",
                      name=f"ya_{sname}")
        yb = act.tile([128, len(kto) * npx], CONV_DT, tag=f"{sname}_yb",
                      name=f"yb_{sname}")
        cd = carry_dram(k, f"carry_{sname}", len(kto) * npx)
        nlim = k.H.get(f'nblk_{sname}')
        if nlim:
            blocks_geo = blocks_geo[:nlim]
        for bi, geo in enumerate(blocks_geo):
            ytile = ya if bi % 2 == 0 else yb
            pb, ccs, hsc = geoms[(geo['inRows'], geo['inW'], geo['d'])]
            ci, co = ccs[bi % 2]
            res = dict(pb=pb, c2o=c2o, resid=resid, hsc=hsc)
            emit_block(k, sname, bi, geo, x, ytile, wpool, bt, ci, co, hidx, res)
            x = ytile

        if feats_name is not None:
            if CONV_DT == FP32:
                nc.sync.dma_start(io[feats_name][:], x[:])
            else:
                f = act.tile([128, len(kto) * npx], FP32, tag=f"{sname}_f",
                             name=f"f_{sname}")
                nc.vector.tensor_copy(f[:], x[:])
                nc.sync.dma_start(io[feats_name][:], f[:])
        nc.sync.dma_start(cd[:], x[:])
        if sname in DEBUG:
            dbg_out(k, sname, x[:], (128, len(kto) * npx))
    return f"carry_{sname}"


def emit_gat(k, nf0):
    """nf0: [128, 1280] fp32 channel-major full-image node features."""
    nc, H, io = k.nc, k.H, k.io
    NSLOT = H['NSLOT']
    NCH = 3 * NSLOT
    gact = k.pool("gat", bufs=1)
    wg = load_plane(k, "gat_w", "w_gat", FP32)
    eye = load_plane(k, "consts", "eye", FP32)
    St = load_plane(k, "gat_w", "S", FP32)
    gidx = load_plane(k, "consts", "gidx", INT32)

    table = nc.dram_tensor("gtable", [N_NODES, TBLW], FP32)
    nc.sync.dma_start(table[:, TBL_GM:TBL_GM + 3], io['gm'][:])

    ag_prev = None   # DRAM [4, 384, F_prev] from previous layer
    fout_prev = None
    for li, (inn, ine, on, oe, h) in enumerate(GAT_CFG):
        Mw = on * h
        Mb = Mw + 2 * h
        Fs = Mw + h
        Fg = TBLW  # full-row gathers (indirect coef = row width)
        ktn = tiles_of(inn)
        wfij = H['gat_wfij'][li]      # [h, 3]
        attn = H['gat_attn'][li]      # [h]
        gbias = H['gat_bias'][li]     # [h]

        # ---- build P table rows
        if li == 0:
            for t in range(10):
                ps = k.psum(Mb)
                wo = H['wp_gat'].index[f'g0.0'][0]
                nc.tensor.matmul(ps[:, :], nf0[:, t * 128:(t + 1) * 128],
                                 wg[:128, wo:wo + Mb], start=True, stop=True)
                pt = gact.tile([128, Mb], FP32, tag="ptile")
                nc.vector.tensor_copy(pt[:], ps[:, :Mb])
                nc.sync.dma_start(table[t * 128:(t + 1) * 128, 0:Mb], pt[:])
        else:
            F_prev = GAT_CFG[li - 1][2] * GAT_CFG[li - 1][4]
            ktp = tiles_of(inn)
            assert inn == F_prev
            for g in range(4):
                for rc in range(3):
                    vs = 128 if rc < 2 else 64
                    nft = gact.tile([128, inn], FP32, tag="nft")
                    nc.sync.dma_start(nft[:vs], ag_prev[g, rc * 128:rc * 128 + vs, :])
                    ps = k.psum(Mb)
                    for ki, ks in enumerate(ktp):
                        tp = k.psum(128)
                        nc.tensor.transpose(tp[:ks, :vs], nft[:vs, ki * 128:ki * 128 + ks],
                                            eye[:vs, :vs])
                        nfT = gact.tile([128, 128], FP32, tag="nfT")
                        nc.vector.tensor_copy(nfT[:ks, :vs], tp[:ks, :vs])
                        wo = H['wp_gat'].index[f'g{li}.{ki}'][0]
                        nc.tensor.matmul(ps[:vs, :], nfT[:ks, :vs],
                                         wg[:ks, wo:wo + Mb],
                                         start=(ki == 0), stop=(ki == len(ktp) - 1))
                    pt = gact.tile([128, Mb], FP32, tag="ptile")
                    nc.vector.tensor_copy(pt[:vs], ps[:vs, :Mb])
                    nc.sync.dma_start(table[g * 320 + rc * 128: g * 320 + rc * 128 + vs,
                                            0:Mb], pt[:vs])

        # ---- gathers
        SRC = gact.tile([128, NCH * Fg], FP32, tag="SRC")
        DST = gact.tile([128, NCH * Fg], FP32, tag="DST")
        for c in range(NCH):
            nc.gpsimd.indirect_dma_start(
                SRC[:, c * Fg:(c + 1) * Fg], None, table[:, :Fg],
                bass.IndirectOffsetOnAxis(ap=gidx[:, c:c + 1], axis=0))
            nc.gpsimd.indirect_dma_start(
                DST[:, c * Fg:(c + 1) * Fg], None, table[:, :Fg],
                bass.IndirectOffsetOnAxis(ap=gidx[:, NCH + c:NCH + c + 1], axis=0))
        SRCv = SRC.rearrange("p (c f) -> p c f", f=Fg)
        DSTv = DST.rearrange("p (c f) -> p c f", f=Fg)

        # ---- edge features
        if li == 0:
            EF = gact.tile([128, NCH * 3], FP32, tag="EF")
            EFv = EF.rearrange("p (c f) -> p c f", f=3)
            nc.vector.tensor_tensor(out=EFv, in0=SRCv[:, :, TBL_GM:TBL_GM + 3],
                                    in1=DSTv[:, :, TBL_GM:TBL_GM + 3], op=ALU.subtract)
            nc.scalar.activation(EFv, EFv, AF.Abs)
        else:
            EF, EFv = fout_prev, fout_prev.rearrange("p (c f) -> p c f",
                                                     f=GAT_CFG[li - 1][4])

        # ---- fout = Lrelu(P_ni[src] + P_nj[dst] + ef @ wfij.T + b)
        FOUT = gact.tile([128, NCH * h], FP32, tag=f"FOUT{li % 2}")
        FOUTv = FOUT.rearrange("p (c f) -> p c f", f=h)
        TMP = gact.tile([128, NCH], FP32, tag="TMP")
        for j in range(h):
            fj = FOUTv[:, :, j]
            nc.vector.tensor_tensor(out=fj, in0=SRCv[:, :, Mw + j],
                                    in1=DSTv[:, :, Mw + h + j], op=ALU.add)
            for kk in range(3):
                nc.vector.tensor_scalar_mul(TMP[:, :NCH], EFv[:, :, kk],
                                            float(wfij[j, kk]))
                nc.vector.tensor_tensor(out=fj, in0=fj, in1=TMP[:, :NCH], op=ALU.add)
            nc.vector.tensor_scalar_add(fj, fj, float(gbias[j]))
            nc.vector.tensor_scalar_mul(TMP[:, :NCH], fj, 0.2)
            nc.vector.tensor_tensor(out=fj, in0=fj, in1=TMP[:, :NCH], op=ALU.max)

        # ---- ex = exp(fout * attn)  (global-shift-free softmax numerator)
        EX = gact.tile([128, NCH * h], FP32, tag="EX")
        EXv = EX.rearrange("p (c f) -> p c f", f=h)
        for j in range(h):
            nc.scalar.activation(EXv[:, :, j], FOUTv[:, :, j], AF.Exp,
                                 scale=float(attn[j]))

        # ---- weighted = [P_node[src] * ex | ex]
        WTD = gact.tile([128, NCH * Fs], FP32, tag="WTD")
        WTDv = WTD.rearrange("p (c f) -> p c f", f=Fs)
        for c in range(NCH):
            for j in range(h):
                nc.vector.tensor_scalar_mul(
                    WTD[:, c * Fs + j * on: c * Fs + (j + 1) * on],
                    SRC[:, c * Fg + j * on: c * Fg + (j + 1) * on],
                    EX[:, c * h + j: c * h + j + 1])
        nc.vector.tensor_copy(WTDv[:, :, Mw:Mw + h], EXv)

        # ---- scatter per node-tile + normalize
        ccg_in = nc.dram_tensor(f"ccg{li}_in", [384, Mw], FP32)
        ccg_out = nc.dram_tensor(f"ccg{li}_out", [4, 384, Mw], FP32)
        for nt in range(3):
            ps = k.psum(Fs)
            for s in range(NSLOT):
                c = nt * NSLOT + s
                nc.tensor.matmul(ps[:, :], St[:, c * 128:(c + 1) * 128],
                                 WTD[:, c * Fs:(c + 1) * Fs],
                                 start=(s == 0), stop=(s == NSLOT - 1))
            NF = gact.tile([128, Mw], FP32, tag="NF")
            DEN = gact.tile([128, h], FP32, tag="DEN")
            nc.vector.tensor_scalar_max(DEN[:], ps[:, Mw:Mw + h], 1e-30)
            nc.vector.reciprocal(DEN[:], DEN[:])
            for j in range(h):
                nc.vector.tensor_scalar_mul(NF[:, j * on:(j + 1) * on],
                                            ps[:, j * on:(j + 1) * on],
                                            DEN[:, j:j + 1])
            nc.sync.dma_start(ccg_in[nt * 128:(nt + 1) * 128, :], NF[:])
        nc.gpsimd.collective_compute("AllGather", ALU.bypass, replica_groups=RG,
                                     ins=[ccg_in[:]], outs=[ccg_out[:]])
        if "gat" in DEBUG:
            dbg_out(k, f"srcg{li}", SRC[:], (128, NCH * Fg))
            dbg_out(k, f"dstg{li}", DST[:], (128, NCH * Fg))
            dbg_out(k, f"ef{li}", EF[:], (128, NCH * (3 if li == 0 else GAT_CFG[li-1][4])))
            ot = nc.dram_tensor(f"dbg_tbl{li}", [N_NODES, TBLW], FP32,
                                kind="ExternalOutput")
            nc.sync.dma_start(ot[:], table[:])
            dbg_out(k, f"fout{li}", FOUT[:], (128, NCH * h))
            dbg_out(k, f"ex{li}", EX[:], (128, NCH * h))
            oag = nc.dram_tensor(f"dbg_agg{li}", [4, 384, Mw], FP32,
                                 kind="ExternalOutput")
            nc.sync.dma_start(oag.rearrange("g n f -> (g n f)"),
                              ccg_out.rearrange("g n f -> (g n f)"))
        ag_prev = ccg_out
        fout_prev = FOUT

    # ---- assemble compact [1280, 32] node output, cls conv
    nfc = nc.dram_tensor("nfc", [N_NODES, 32], FP32)
    for g in range(4):
        for rc in range(3):
            vs = 128 if rc < 2 else 64
            nc.sync.dma_start(nfc[g * 320 + rc * 128: g * 320 + rc * 128 + vs, :],
                              ag_prev[g, rc * 128:rc * 128 + vs, :])
    lastr = gact.tile([128, 1280], FP32, tag="lastr")
    nc.sync.dma_start(lastr[:32], nfc.rearrange("(a b) c -> a (b c)", a=32))
    wo = H['wp_gat'].index['cls'][0]
    bcls = H['gat_cls_b']  # np [128]
    bct = k.pool("consts").tile([128, 1], FP32, tag="bcls")
    nc.sync.dma_start(bct[:], k.io['b_cls'][:])
    last = gact.tile([128, 1280], FP32, tag="last")
    for c0 in range(0, 1280, 512):
        n = min(512, 1280 - c0)
        ps = k.psum(n)
        nc.tensor.matmul(ps[:, :], wg[:32, wo:wo + 128], lastr[:32, c0:c0 + n],
                         start=True, stop=True)
        nc.scalar.activation(last[:, c0:c0 + n], ps[:, :n], AF.Identity, bias=bct[:, 0:1])
    nc.sync.dma_start(k.io['last'][:], last[:])


# ================================================================ build

def build_program(H):
    nc = bacc.Bacc("TRN2", target_bir_lowering=False, debug=False, num_devices=N_CORES)
    io = {}

    def inp(name, shape, dt):
        io[name] = nc.dram_tensor(name, list(shape), dt, kind="ExternalInput")

    def outp(name, shape, dt=FP32):
        io[name] = nc.dram_tensor(name, list(shape), dt, kind="ExternalOutput")

    inp("x27", (27, 71 * 320), CONV_DT)
    inp("masks", (128, 8), FP32)
    inp("w_stem", (128, H['wp_stem'].len), CONV_DT)
    inp("b_stem", (128, H['bp_stem'].len), FP32)
    for sn in ("s1", "s2", "s3", "s4"):
        inp(f"w_{sn}", (128, H[f'wlen_{sn}'] * H[f'nb_{sn}']), CONV_DT)
        inp(f"b_{sn}", (128, H[f'bp_{sn}'].len), FP32)
        inp(f"hidx_{sn}", (128, 2 * H[f'ktm_{sn}']), INT32)
    inp("w_red", (128, H['wp_red'].len), CONV_DT)
    inp("w_gat", (128, H['wp_gat'].len), FP32)
    inp("b_cls", (128, 1), FP32)
    inp("S", (128, 3 * H['NSLOT'] * 128), FP32)
    inp("gidx", (128, 2 * 3 * H['NSLOT']), INT32)
    inp("gm", (N_NODES, 3), FP32)
    inp("eye", (128, 128), FP32)

    outp("feats0", (128, 32 * 160))
    outp("feats1", (128, 2 * 16 * 80))
    outp("feats2", (128, 4 * 8 * 40))
    outp("feats3", (128, 8 * 8 * 40))
    outp("last", (128, 1280))

    order = ["stem", "s1", "s2", "s3", "s4", "red", "gat", "all"]
    lim = order.index(UPTO)
    done = ["feats0"]

    with ExitStack() as ctx:
        tc = ctx.enter_context(tile.TileContext(nc))
        k = K(nc, tc, ctx, H, io)
        for pn, bf, sp in (("consts", 1, "SBUF"), ("psum", 8, "PSUM"),
                           ("dbgp", 2, "SBUF")):
            k.pool(pn, bufs=bf, space=sp)
        pooled = emit_stem(k)
        while lim > 0:
            s1 = [dict(Cin=128 if b == 0 else 256, Cm=64, Cout=256, d=1, W=80,
                       rows=16, inW=80, inRows=16, stride=1, first=(b == 0))
                  for b in range(3)]
            s2 = [dict(Cin=256 if b == 0 else 512, Cm=128, Cout=512, d=1,
                       W=40, rows=8, inW=80 if b == 0 else 40,
                       inRows=16 if b == 0 else 8, stride=2 if b == 0 else 1,
                       first=(b == 0)) for b in range(4)]
            s3 = [dict(Cin=512 if b == 0 else 1024, Cm=256, Cout=1024, d=2, W=40,
                       rows=8, inW=40, inRows=8, stride=1, first=(b == 0))
                  for b in range(23)]
            s4 = [dict(Cin=1024 if b == 0 else 2048, Cm=512, Cout=2048, d=4, W=40,
                       rows=8, inW=40, inRows=8, stride=1, first=(b == 0))
                  for b in range(3)]
            y = emit_stage(k, "s1", s1, pooled, "feats1")  # y is carry name
            done.append("feats1")
            if lim <= order.index("s1"):
                break
            y = emit_stage(k, "s2", s2, y, "feats2")
            done.append("feats2")
            if lim <= order.index("s2"):
                break
            y = emit_stage(k, "s3", s3, y, "feats3")
            done.append("feats3")
            if lim <= order.index("s3"):
                break
            y = emit_stage(k, "s4", s4, y)
            if lim <= order.index("s4"):
                break
            # red conv -> nf slice -> AG -> nf0 [128, 1280]
            wr = load_plane(k, "gat_w", "w_red", CONV_DT)
            y4 = k.pool("gat", bufs=1).tile([128, 16 * 320], CONV_DT, tag="y4",
                                            name="y4")
            nc.sync.dma_start(y4[:], k.io[y][:])
            nf_my = k.pool("gat", bufs=1).tile([128, 320], FP32, tag="nf_my",
                                               name="nf_my")
            wpr = H['wp_red']
            ps = k.psum(320)
            for ki in range(16):
                wo, wk, wm = wpr.index[f'red.{ki}.0']
                nc.tensor.matmul(ps[:, :], wr[:wk, wo:wo + wm],
                                 y4[:128, ki * 320:(ki + 1) * 320],
                                 start=(ki == 0), stop=(ki == 15))
            nc.vector.tensor_copy(nf_my[:], ps[:, :320])
            ccr_in = nc.dram_tensor("ccr_in", [128, 320], FP32)
            ccr_out = nc.dram_tensor("ccr_out", [4, 128, 320], FP32)
            nc.sync.dma_start(ccr_in[:], nf_my[:])
            nc.gpsimd.collective_compute("AllGather", ALU.bypass, replica_groups=RG,
                                         ins=[ccr_in[:]], outs=[ccr_out[:]])
            nf0 = k.pool("gat", bufs=1).tile([128, 1280], FP32, tag="nf0")
            nc.sync.dma_start(nf0[:], ccr_out.rearrange("g p f -> p g f"))
            if "nf0" in DEBUG:
                dbg_out(k, "nf0", nf0[:], (128, 1280))
            if lim <= order.index("red"):
                break
            emit_gat(k, nf0)
            done.append("last")
            break
        _finish_dummy(k, [n for n in ("feats1", "feats2", "feats3", "last")
                          if n not in done])

    nc.finalize()
    return nc


def _finish_dummy(k, names):
    """write zeros to unused outputs so every declared output is produced"""
    nc = k.nc
    for n in names:
        shape = list(k.io[n].shape)
        t = k.pool("dbgp", bufs=2).tile(shape, FP32, tag="dbg")
        nc.gpsimd.memset(t[:], 0.0)
        nc.sync.dma_start(k.io[n][:], t[:])


# ================================================================ host side

def prep_host(params, x, final_x, src, dst):
    """Build all host-side constants. Returns H (shared) and per-core input maps."""
    H = {}
    rp = params['resnet']

    # ---- stem pack
    wp, bp = Plane(CONV_DT), Plane(FP32)
    w1, b1 = fold_bn(rp['c1'], rp['b1'])
    wp.add('c1', w1.transpose(2, 3, 1, 0).reshape(27, 64))
    bp_add(bp, 'c1', b1, [64])
    w2, b2 = fold_bn(rp['c2'], rp['b2'])
    for dy in range(3):
        for dx in range(3):
            wp.add(f'c2.{dy}{dx}', w2[:, :, dy, dx].T)
    bp_add(bp, 'c2', b2, [64])
    w3, b3 = fold_bn(rp['c3'], rp['b3'])
    for dy in range(3):
        for dx in range(3):
            wp.add(f'c3.{dy}{dx}', w3[:, :, dy, dx].T)
    bp_add(bp, 'c3', b3, [128])
    H['wp_stem'], H['bp_stem'] = wp, bp

    # ---- stage packs (per-block planes concatenated)
    for si, (planes, nb, stride, dil) in enumerate(STAGES):
        sname = f"s{si + 1}"
        bp = Plane(FP32)
        block_planes = []
        wlen = None
        for bi, blk in enumerate(rp['layers'][si]):
            wpb = Plane(CONV_DT)
            name = f"{sname}b{bi}"
            wc1, bc1 = fold_bn(blk['w1'], blk['bn1'])
            pack_1x1(wpb, bp, f"{name}.c1", wc1[:, :, 0, 0], bc1)
            wc2, bc2 = fold_bn(blk['w2'], blk['bn2'])
            pack_3x3(wpb, bp, f"{name}.c2", wc2, bc2)
            wc3, bc3 = fold_bn(blk['w3'], blk['bn3'])
            pack_1x1(wpb, bp, f"{name}.c3", wc3[:, :, 0, 0], bc3)
            if 'dw' in blk:
                wd, bd = fold_bn(blk['dw'], blk['dbn'])
                pack_1x1(wpb, bp, f"{name}.ds", wd[:, :, 0, 0], bd)
            block_planes.append(wpb)
        wlen = max(p.len for p in block_planes)
        # pad all blocks to wlen and merge indexes with block offsets
        wp = Plane(CONV_DT)
        planes_np = []
        for bi, p in enumerate(block_planes):
            arr = p.plane()
            if arr.shape[1] < wlen:
                arr = np.concatenate(
                    [arr, np.zeros((128, wlen - arr.shape[1]), arr.dtype)], axis=1)
            planes_np.append(arr)
            for nm, (off, Kd, M) in p.index.items():
                wp.index[nm] = (off, Kd, M)   # offsets are block-relative
            for nm, v in p.gindex.items():
                wp.gindex[nm] = v
        H[f'wp_{sname}'] = wp
        H[f'bp_{sname}'] = bp
        H[f'wlen_{sname}'] = wlen
        H[f'nb_{sname}'] = nb
        H[f'ktm_{sname}'] = len(tiles_of(planes))
        H[f'wplane_{sname}'] = np.concatenate(planes_np, axis=1)

    # ---- red conv
    wp = Plane(CONV_DT)
    wred = _np(params['red'])[:, :, 0, 0]   # [128, 2048]
    pack_1x1(wp, Plane(FP32), 'red', wred, None)
    H['wp_red'] = wp

    # ---- GAT + cls pack (fp32)
    wp = Plane(FP32)
    gp = params['gat']
    H['gat_wfij'], H['gat_attn'], H['gat_bias'] = [], [], []
    for li, (inn, ine, on, oe, h) in enumerate(GAT_CFG):
        g = gp[li]
        WbT = np.concatenate([_np(g['w_node']).T, _np(g['w_ni']).T,
                              _np(g['w_nj']).T], axis=1)  # [inn, Mb]
        for ki, ks in enumerate(tiles_of(inn)):
            wp.add(f'g{li}.{ki}', WbT[ki * 128:ki * 128 + ks, :])
        H['gat_wfij'].append(_np(g['w_fij']).astype(np.float32))
        H['gat_attn'].append(_np(g['attn']).reshape(h).astype(np.float32))
        H['gat_bias'].append(_np(g['bias']).reshape(h).astype(np.float32))
    wp.add('cls', _np(params['cls_w'])[:, :, 0, 0].T)   # [32, 128]
    H['wp_gat'] = wp
    bcls = np.zeros((128, 1), np.float32)
    bcls[:, 0] = _np(params['cls_b'])
    H['gat_cls_b'] = bcls

    # ---- graph structures
    src_n, dst_n = _np(src).astype(np.int64), _np(dst).astype(np.int64)
    order = np.argsort(dst_n, kind='stable')
    NT = 10  # global node tiles (128 each)
    # per (quarter, local tile 0..2): edges
    edges_by_tile = [[[] for _ in range(3)] for _ in range(4)]
    for e in order:
        d_ = dst_n[e]
        q, loc = divmod(d_, 320)
        edges_by_tile[q][loc // 128].append(e)
    NSLOT = max(1, max((len(v) + 127) // 128
                       for qv in edges_by_tile for v in qv))
    H['NSLOT'] = NSLOT
    NCH = 3 * NSLOT
    EPQ = NCH * 128
    # per quarter arrays
    gidx_q, S_q = [], []
    for q in range(4):
        gi = np.zeros((128, 2 * NCH), np.int32)
        S = np.zeros((3, NSLOT, 128, 128), np.float32)
        for nt in range(3):
            ev = edges_by_tile[q][nt]
            for i, e in enumerate(ev):
                s_, p_ = divmod(i, 128)
                c = nt * NSLOT + s_
                gi[p_, c] = src_n[e]
                gi[p_, NCH + c] = dst_n[e]
                S[nt, s_, p_, dst_n[e] - 320 * q - nt * 128] = 1.0
        gidx_q.append(gi)
        # S device layout: [128 edges, (nt, slot, node)]
        S_q.append(S.transpose(2, 0, 1, 3).reshape(128, 3 * NSLOT * 128))

    # ---- per-core inputs
    xf = _np(x).astype(np.float32)
    gmf = _np(final_x).astype(np.float32)
    B = xf.shape[0]
    in_maps = []
    shared = {
        'w_stem': H['wp_stem'].plane(), 'b_stem': H['bp_stem'].plane(),
        'w_red': H['wp_red'].plane(), 'w_gat': H['wp_gat'].plane(),
        'b_cls': H['gat_cls_b'], 'eye': np.eye(128, dtype=np.float32),
    }
    for s in ('s1', 's2', 's3', 's4'):
        shared[f'w_{s}'] = H[f'wplane_{s}']
        shared[f'b_{s}'] = H[f'bp_{s}'].plane()

    for core in range(N_CORES):
        g, q = divmod(core, 4)
        m = dict(shared)
        # x27 im2col
        x27 = np.zeros((27, 71, 320), np_dt(CONV_DT))
        xg = xf[g]
        for dy in range(3):
            for dx in range(3):
                for c in range(3):
                    p = (dy * 3 + dx) * 3 + c
                    for r in range(71):
                        gr = 64 * q - 3 + r + 2 * (dy - 1)
                        if 0 <= gr < 256:
                            cl = 2 * (dx - 1)
                            lo, hi = max(0, -cl), min(320, 320 - cl)
                            x27[p, r, lo:hi] = xg[c, gr, lo + cl:hi + cl]
        m['x27'] = x27.reshape(27, 71 * 320)
        # masks
        mk = np.ones((128, 8), np.float32)
        if q == 0:
            mk[:, 0] = 0; mk[:, 1] = 0
        if q == 3:
            mk[:, 2] = 0; mk[:, 3] = 0; mk[:, 4] = 0
        m['masks'] = mk
        # halo gather indices per stage
        for si, (planes, nb, stride, dil) in enumerate(STAGES):
            sname = f"s{si + 1}"
            Cm = planes
            KT = len(tiles_of(Cm))
            hi = np.full((128, 2 * KT), SENT, np.int32)
            for kt, ks in enumerate(tiles_of(Cm)):
                for p in range(ks):
                    ch = kt * 128 + p
                    if q > 0:
                        hi[p, 2 * kt] = (q - 1) * Cm * 2 + ch * 2 + 1
                    if q < 3:
                        hi[p, 2 * kt + 1] = (q + 1) * Cm * 2 + ch * 2 + 0
            m[f'hidx_{sname}'] = hi
        m['gidx'] = gidx_q[q]
        m['S'] = S_q[q]
        m['gm'] = gmf[g]
        in_maps.append(m)
    return H, in_maps


def kernel(params, x, final_x, src, dst):
    H, in_maps = prep_host(params, x, final_x, src, dst)
    nc = build_program(H)
    res = run_bass_kernel_spmd(nc, in_maps, list(range(N_CORES)))
    return assemble(res, H)


def assemble(res, H):
    B = 2
    f0 = np.zeros((B, 128, 128, 160), np.float32)
    f1 = np.zeros((B, 256, 64, 80), np.float32)
    f2 = np.zeros((B, 512, 32, 40), np.float32)
    f3 = np.zeros((B, 1024, 32, 40), np.float32)
    last = np.zeros((B, 128, 32, 40), np.float32)
    for core in range(N_CORES):
        g, q = divmod(core, 4)
        r = res.results[core]
        f0[g, :, 32 * q:32 * q + 32, :] = r['feats0'].reshape(128, 32, 160)
        a = r['feats1'].reshape(128, 2, 16, 80)
        for kt in range(2):
            f1[g, kt * 128:(kt + 1) * 128, 16 * q:16 * q + 16, :] = a[:, kt]
        a = r['feats2'].reshape(128, 4, 8, 40)
        for kt in range(4):
            f2[g, kt * 128:(kt + 1) * 128, 8 * q:8 * q + 8, :] = a[:, kt]
        a = r['feats3'].reshape(128, 8, 8, 40)
        for kt in range(8):
            f3[g, kt * 128:(kt + 1) * 128, 8 * q:8 * q + 8, :] = a[:, kt]
        if q == 0:
            last[g] = r['last'].reshape(128, 32, 40)
    return f0, f1, f2, f3, last
